# revision 2
# baseline (speedup 1.0000x reference)
"""Multi-head attention (B=2, S=2048, D=1024, H=16) on 8 TRN2 NeuronCores, v2.

Sharding: 2-way batch data-parallel x 4-way head tensor-parallel.
Core c handles batch c//4 with heads [4r, 4r+4) where r = c%4.

Key ideas vs v1:
- bf16/fp8 activations+weights (less HBM traffic, same-or-better PE rate).
- key compaction without the +1 safety block (exact ceil(n/128) blocks).
- transposed PV (attn^T is exactly the exp output layout): ctx comes out
  [q, d] with full 128-partition outputs -> half the PE rows of v1's PV,
  and softmax normalization becomes a cheap per-partition scalar multiply.
- denominator via an all-ones column appended to each head's v tile.
- 1024-wide exp (2 heads x 512 q per Activation instruction, reading a
  double-buffered 2-bank PSUM tile).
- fp8(e4m3) DoubleRow projections: the host ships x and W pre-interleaved
  as [128, 2-slot, .] with contraction dim d = 256*g + 128*slot + p, so a
  256-deep contraction runs at 0.5 cycles/row with no on-device shuffle.
- optional fp8 output projection: ctxT's [dim-pair, token] layout is
  already DoubleRow-compatible (slot = head-pair index).
- AllToAll in 4 per-query-block rounds, each core taking a 128-token
  slice of every block, so output projection pipelines behind attention
  on every core symmetrically.
"""
import math

import numpy as np

import concourse.mybir as mybir
import concourse.tile as tile
from concourse import bacc, bass_utils

B, S, D, H = 2, 2048, 1024, 16
DEPTH = 64
N_CORES = 8
GROUP = 4              # cores per batch (tensor parallel over heads)
HPC = H // GROUP       # 4 heads per core
HL = HPC * DEPTH       # 256 local head dims
NDT = D // 128         # 8 contraction tiles of 128
NDP = NDT // 2         # 4 double-row contraction tiles of 256
NQB = S // 512         # 4 query blocks of 512 per batch

F32 = mybir.dt.float32
BF16 = mybir.dt.bfloat16
F8 = mybir.dt.float8e4
AFT = mybir.ActivationFunctionType
DR = mybir.MatmulPerfMode.DoubleRow


def build_nc(kblocks: int = 8, proj_fp8: bool = False, out_fp8: bool = False,
             collective: bool = True, num_devices: int = N_CORES,
             dump: bool = False, reps: int = 1):
    SK = kblocks * 128
    NKT = SK // 128
    xdt = F8 if proj_fp8 else BF16
    cdt = F8 if out_fp8 else BF16
    nc = bacc.Bacc(
        "TRN2", target_bir_lowering=False, debug=False, num_devices=num_devices
    )

    # ---- I/O (per-core slices prepared by the host) ----
    # x layouts are chunk-major so one chunk = one 2-dim DMA:
    #   fp8:  [nch, 128, NDP, 2, W]; row d = 256*g + 128*slot + p
    #   bf16: [nch, 128, NDT, W]
    # chunk widths: xk 256, xq 512, xv 512.
    KW, QW, VW = min(256, SK), 512, min(512, SK)
    NKC, NQC, NVC = SK // KW, S // QW, SK // VW
    xin = ([NDP, 2] if proj_fp8 else [NDT])
    xq = nc.dram_tensor("xq", [NQC, 128] + xin + [QW], xdt, kind="ExternalInput")
    xk = nc.dram_tensor("xk", [NKC, 128] + xin + [KW], xdt, kind="ExternalInput")
    xv = nc.dram_tensor("xv", [NVC, 128] + xin + [VW], xdt, kind="ExternalInput")
    wshape = ([NDP, 128, 2] if proj_fp8 else [NDT, 128])
    wq = nc.dram_tensor("wq", wshape + [HL], xdt, kind="ExternalInput")
    wk = nc.dram_tensor("wk", wshape + [HL], xdt, kind="ExternalInput")
    wv = nc.dram_tensor("wv", wshape + [HL], xdt, kind="ExternalInput")
    # wo: fp8 [GROUP*2, 128, 2, D] with row (i, slot, p) = ctx dim
    # 256i + 128*slot + p; bf16 [NDT, 128, D] plain.
    woshape = ([GROUP, 128, 2] if out_fp8 else [NDT, 128])
    wo = nc.dram_tensor("wo", woshape + [D], cdt, kind="ExternalInput")
    # consts packed in one tensor: bk | bq | maskb | bvb (f32 columns)
    NCC = 4 + kblocks + HPC * 65
    consts = nc.dram_tensor("consts", [128, NCC], F32, kind="ExternalInput")
    bob = nc.dram_tensor("bob", [128, D], F32, kind="ExternalInput")
    ident = nc.dram_tensor("ident", [128, 128], BF16, kind="ExternalInput")
    out = nc.dram_tensor("out", [NQB, 128, D], BF16, kind="ExternalOutput")

    if dump:
        d_qT = nc.dram_tensor("d_qT", [128, 2, S], BF16, kind="ExternalOutput")
        d_kT = nc.dram_tensor("d_kT", [128, 2, SK], BF16, kind="ExternalOutput")
        d_v = nc.dram_tensor("d_v", [128, NKT, HPC * 65], BF16,
                             kind="ExternalOutput")
        d_e = nc.dram_tensor("d_e", [128, kblocks, HPC * 512], BF16,
                             kind="ExternalOutput")

    # global 8-way AllToAll: chunk g = my 256 dims for tokens
    # [qb*512 + g*64, +64) of my batch; core c ends up with all 1024 dims of
    # BOTH batches' 64-token slice c.
    a2a_in = [nc.dram_tensor(f"a2a_in{j}", [N_CORES, 128, 2, 64], cdt)
              for j in range(NQB)]
    a2a_out = [nc.dram_tensor(f"a2a_out{j}", [N_CORES, 128, 2, 64], cdt)
               for j in range(NQB)]

    with tile.TileContext(nc) as tc:
        with (
            tc.tile_pool(name="w", bufs=1) as wp,
            tc.tile_pool(name="big", bufs=1) as bigp,
            tc.tile_pool(name="io", bufs=4) as iop,
            tc.tile_pool(name="ps", bufs=1, space="PSUM") as ps,
        ):
            # ---- constants ----
            wsl = [NDP, 2] if proj_fp8 else [NDT]
            wq_sb = wp.tile([128] + wsl + [HL], xdt)
            wk_sb = wp.tile([128] + wsl + [HL], xdt)
            wv_sb = wp.tile([128] + wsl + [HL], xdt)
            wre = "g p s m -> p g s m" if proj_fp8 else "g p m -> p g m"
            const_sb = wp.tile([128, NCC], F32)
            bk_sb = const_sb[:, 0:2]
            bq_sb = const_sb[:, 2:4]
            maskb_sb = const_sb[:, 4:4 + kblocks]
            bvb_sb = const_sb[:, 4 + kblocks:4 + kblocks + HPC * 65]
            ident_sb = wp.tile([128, 128], BF16)
            # K-path constants first so the first exp fires as early as
            # possible; wq/wv/remaining consts are queued mid-schedule.
            nc.sync.dma_start(const_sb[:], consts.ap())
            nc.sync.dma_start(wk_sb[:], wk.ap().rearrange(wre))

            # ---- persistent activations (one tile per input chunk) ----
            xsl = [NDP, 2] if proj_fp8 else [NDT]
            xq_sb = [bigp.tile([128] + xsl + [QW], xdt, name=f"xq{i}")
                     for i in range(NQC)]
            xk_sb = [bigp.tile([128] + xsl + [KW], xdt, name=f"xk{i}")
                     for i in range(NKC)]
            xv_sb = [bigp.tile([128] + xsl + [VW], xdt, name=f"xv{i}")
                     for i in range(NVC)]
            qT = bigp.tile([128, 2, S], BF16)    # local q dim j*128+p
            kT = bigp.tile([128, 2, SK], BF16)
            v_sb = bigp.tile([128, NKT, HPC * 65], BF16)
            e_sb = bigp.tile([128, 2, kblocks, HPC * 512], BF16)

            # ones columns (denominator trick): v_sb[:, :, h*65+64] = 1
            nc.vector.memset(
                v_sb[:].rearrange("p t (h c) -> p t h c", h=HPC)[:, :, :, 64], 1.0
            )

            # warm the Exp activation table off the critical path
            warm = iop.tile([1, 1], F32, tag="warm", bufs=1, name="warm")
            nc.scalar.activation(warm[:], warm[:], AFT.Exp)

            # ---------- emission helpers ----------
            def load_chunk(x_sb, x_dram, c):
                nc.sync.dma_start(x_sb[c][:], x_dram.ap()[c])

            NG = NDP if proj_fp8 else NDT

            def qk_proj(x_t, w_sb, b_sb, dst, cc, w):
                """project chunk tile x_t -> dst[:, j, cc:cc+w]."""
                for j in range(2):
                    ps_p = ps.tile([128, 512], F32, tag="mm", bufs=2, name="ps_p")
                    for g in range(NG):
                        if proj_fp8:
                            nc.tensor.matmul(
                                ps_p[:, 0:w],
                                w_sb[:, g, :, j * 128:(j + 1) * 128],
                                x_t[:, g, :, 0:w],
                                perf_mode=DR,
                                start=(g == 0), stop=(g == NG - 1),
                            )
                        else:
                            nc.tensor.matmul(
                                ps_p[:, 0:w], w_sb[:, g, j * 128:(j + 1) * 128],
                                x_t[:, g, 0:w],
                                start=(g == 0), stop=(g == NG - 1),
                            )
                    nc.vector.tensor_scalar_add(
                        dst[:, j, cc:cc + w], ps_p[:, 0:w], b_sb[:, j:j + 1]
                    )

            def v_proj(kt):
                """value projection for key tile kt -> v_sb[:, kt, :]."""
                x_t = xv_sb[(kt * 128) // VW]
                c0 = (kt * 128) % VW
                ps_v = ps.tile([128, 512], F32, tag="mm", bufs=2, name="ps_v")
                for g in range(NG):
                    if proj_fp8:
                        nc.tensor.matmul(
                            ps_v[:, 0:HL],
                            x_t[:, g, :, c0:c0 + 128],
                            wv_sb[:, g, :, :],
                            perf_mode=DR,
                            start=(g == 0), stop=(g == NG - 1),
                        )
                    else:
                        nc.tensor.matmul(
                            ps_v[:, 0:HL], x_t[:, g, c0:c0 + 128],
                            wv_sb[:, g, :],
                            start=(g == 0), stop=(g == NG - 1),
                        )
                pv4 = ps_v[:, 0:HL].rearrange("p (h c) -> p h c", h=HPC)
                vb4 = bvb_sb.rearrange("p (h c) -> p h c", h=HPC)[:, :, 0:64]
                dst = v_sb[:, kt, :].rearrange("p (h c) -> p h c", h=HPC)[:, :, 0:64]
                nc.vector.tensor_add(dst, pv4, vb4)

            def qk_exp(qb, kb):
                """logits + exp, two heads per Activation instruction."""
                for jp in range(2):  # head pair (jp*2, jp*2+1)
                    lg = ps.tile([128, 1024], F32, tag="lg", bufs=2, name="lg")
                    for hh in range(2):
                        h = jp * 2 + hh
                        p0 = (h % 2) * 64
                        nc.tensor.matmul(
                            lg[:, hh * 512:(hh + 1) * 512],
                            kT[p0:p0 + 64, h // 2, kb * 128:(kb + 1) * 128],
                            qT[p0:p0 + 64, h // 2, qb * 512:(qb + 1) * 512],
                            start=True, stop=True,
                        )
                    nc.scalar.activation(
                        e_sb[:, qb % 2, kb, jp * 1024:(jp + 1) * 1024], lg[:],
                        AFT.Exp, bias=maskb_sb[:, kb:kb + 1], scale=0.125,
                    )

            cn_tiles = {}

            def pv_mm(qb, qq):
                """PV matmuls + normalize for one 128-query chunk."""
                cx = ps.tile([128, HPC * 65], F32, tag="cx", bufs=2, name="cx")
                for h in range(HPC):
                    for kb in range(kblocks):
                        nc.tensor.matmul(
                            cx[:, h * 65:(h + 1) * 65],
                            e_sb[:, qb % 2, kb,
                                 h * 512 + qq * 128:h * 512 + (qq + 1) * 128],
                            v_sb[:, kb, h * 65:(h + 1) * 65],
                            start=(kb == 0), stop=(kb == kblocks - 1),
                        )
                rcp = iop.tile([128, 4], F32, tag="rcp", bufs=3, name="rcp")
                dens = cx[:].rearrange("p (h c) -> p h c", h=HPC)[:, :, 64]
                nc.vector.reciprocal(rcp[:], dens)
                for hp in range(2):
                    cn = iop.tile([128, 128], BF16, tag="cn", bufs=8, name="cn")
                    for hh in range(2):
                        h = hp * 2 + hh
                        if qb == 3:
                            # Act is idle after the last exp
                            nc.scalar.activation(
                                cn[:, hh * 64:(hh + 1) * 64],
                                cx[:, h * 65:h * 65 + 64], AFT.Copy,
                                scale=rcp[:, h:h + 1],
                            )
                        else:
                            nc.vector.tensor_scalar_mul(
                                cn[:, hh * 64:(hh + 1) * 64],
                                cx[:, h * 65:h * 65 + 64], rcp[:, h:h + 1],
                            )
                    cn_tiles[(qq, hp)] = cn

            def pv_tr(qb, qqs):
                """transpose + copy + a2a staging for query chunks."""
                for qq in qqs:
                    ctq = iop.tile([128, 2, 128], cdt, tag="ctq", bufs=4,
                                   name="ctq")
                    for hp in range(2):
                        cn = cn_tiles.pop((qq, hp))
                        # transpose via PE into a bitcast view of an mm tile
                        tp = ps.tile([128, 512], F32, tag="mm", bufs=2, name="tp")
                        tpv = tp[:, 0:64].bitcast(BF16)
                        nc.tensor.transpose(tpv, cn[:], ident_sb[:])
                        if qb == 3:
                            # Act engine is idle after the last exp; keep the
                            # tail's mul->transpose->copy chain off the DVE
                            nc.scalar.copy(ctq[:, hp], tpv)
                        else:
                            nc.vector.tensor_copy(ctq[:, hp], tpv)
                    # stage as two 64-token chunks for the exchange
                    for s in range(2):
                        nc.sync.dma_start(
                            a2a_in[qb].ap()[2 * qq + s],
                            ctq[:, :, s * 64:(s + 1) * 64],
                        )

            def a2a_exchange(j, g0=0, g1=N_CORES):
                if collective:
                    assert (g0, g1) == (0, N_CORES)
                    nc.gpsimd.collective_compute(
                        "AllToAll",
                        mybir.AluOpType.bypass,
                        replica_groups=[list(range(N_CORES))],
                        ins=[a2a_in[j].ap().opt()],
                        outs=[a2a_out[j].ap().opt()],
                    )
                else:
                    nc.gpsimd.dma_start(
                        a2a_out[j].ap()[g0:g1], a2a_in[j].ap()[g0:g1]
                    )

            def ctxf_load(j, halves=(0, 1)):
                # free layout (g, hp, b*64+t): each (g, hp) slice is a valid
                # single-free-dim stationary operand covering both batches
                ctxf = iop.tile([128, GROUP, 2, 128], cdt, tag="ctxf", bufs=2,
                                name="ctxf")
                for b in halves:
                    for hp in range(2):
                        nc.sync.dma_start(
                            ctxf[:, :, hp, b * 64:(b + 1) * 64],
                            a2a_out[j].ap()[b * 4:(b + 1) * 4, :, hp].rearrange(
                                "g p t -> p g t"),
                        )
                return ctxf

            def out_proj(j, ctxf, wo_sb, bob_sb):
                # rows 0:64 = my 64-token slice of batch 0, rows 64:128 batch 1
                o_sb = iop.tile([128, D], BF16, tag="osb", bufs=2, name="o_sb")
                # output rows 0:64 are my batch-0 tokens, 64:128 batch-1
                for half in range(2):
                    ps_o = ps.tile([128, 512], F32, tag="mm", bufs=2,
                                   name="ps_o")
                    n = 0
                    for g in range(GROUP):
                        for hp in range(2):
                            nc.tensor.matmul(
                                ps_o[:], ctxf[:, g, hp, :],
                                wo_sb[:, g * 2 + hp,
                                      half * 512:(half + 1) * 512],
                                start=(n == 0), stop=(n == 2 * GROUP - 1),
                            )
                            n += 1
                    nc.vector.tensor_add(
                        o_sb[:, half * 512:(half + 1) * 512], ps_o[:],
                        bob_sb[:, half * 512:(half + 1) * 512],
                    )
                    nc.sync.dma_start(
                        out.ap()[j, :, half * 512:(half + 1) * 512],
                        o_sb[:, half * 512:(half + 1) * 512],
                    )

            # ---------- schedule ----------
            if out_fp8:
                wo_sb = wp.tile([128, GROUP, 2, D], F8)
            else:
                wo_sb = wp.tile([128, NDT, D], BF16)
            bob_sb = wp.tile([128, D], F32)

            for rep in range(reps):
                # ---- round 0: all projections + exp(0) stream.
                # start with one 256-key chunk so exp(0,0) fires early ----
                if rep == 0:
                    nc.sync.dma_start(wq_sb[:], wq.ap().rearrange(wre))
                load_chunk(xk_sb, xk, 0)
                load_chunk(xq_sb, xq, 0)
                qk_proj(xk_sb[0], wk_sb, bk_sb, kT, 0, KW)
                qk_proj(xq_sb[0], wq_sb, bq_sb, qT, 0, 512)
                for kb in range(min(2, kblocks)):
                    qk_exp(0, kb)
                for c in range(1, NKC):
                    load_chunk(xk_sb, xk, c)
                if rep == 0:
                    nc.sync.dma_start(ident_sb[:], ident.ap())
                for c in range(1, NQC):
                    load_chunk(xq_sb, xq, c)
                for c in range(1, NKC):
                    qk_proj(xk_sb[c], wk_sb, bk_sb, kT, c * KW, KW)
                for kb in range(2, min(4, kblocks)):
                    qk_exp(0, kb)
                qk_proj(xq_sb[1], wq_sb, bq_sb, qT, 512, 512)
                for kb in range(4, kblocks):
                    qk_exp(0, kb)
                if rep == 0:
                    nc.sync.dma_start(wv_sb[:], wv.ap().rearrange(wre))
                for c in range(NVC):
                    load_chunk(xv_sb, xv, c)
                for kt in range(min(4, NKT)):
                    v_proj(kt)
                if rep == 0:
                    nc.sync.dma_start(
                        wo_sb[:],
                        wo.ap().rearrange(
                            "g p s m -> p g s m" if out_fp8 else "g p m -> p g m"
                        ),
                    )
                    nc.sync.dma_start(bob_sb[:], bob.ap())

                # ---- rounds 1-3: exp(r) stream hosting round r-1's
                # pv/exchange and round r-2's output projection ----
                ctxf_t = {}
                for r in range(1, NQB):
                    qk_exp(r, 0)
                    if r == 1:
                        for kt in range(4, NKT):
                            v_proj(kt)
                    qk_exp(r, 1)
                    if r >= 2:
                        out_proj(r - 2, ctxf_t.pop(r - 2), wo_sb, bob_sb)
                    if r < NQB - 1:
                        qk_proj(xq_sb[r + 1], wq_sb, bq_sb, qT,
                                (r + 1) * 512, 512)
                    qk_exp(r, 2)
                    pv_mm(r - 1, 0)
                    qk_exp(r, 3)
                    pv_mm(r - 1, 1)
                    pv_tr(r - 1, (0,))
                    qk_exp(r, 4)
                    pv_mm(r - 1, 2)
                    pv_tr(r - 1, (1,))
                    qk_exp(r, 5)
                    pv_mm(r - 1, 3)
                    pv_tr(r - 1, (2,))
                    qk_exp(r, 6)
                    pv_tr(r - 1, (3,))
                    if not collective:
                        a2a_exchange(r - 1, 0, 4)
                    qk_exp(r, 7)
                    if collective:
                        a2a_exchange(r - 1)
                    else:
                        a2a_exchange(r - 1, 4, 8)
                    ctxf_t[r - 1] = ctxf_load(r - 1)

                # ---- tail: last block's pv, then the two outputs ----
                for qq in range(4):
                    pv_mm(3, qq)
                    pv_tr(3, (qq,))
                    if qq == 1 and not collective:
                        a2a_exchange(3, 0, 4)
                out_proj(2, ctxf_t.pop(2), wo_sb, bob_sb)
                if collective:
                    a2a_exchange(3)
                else:
                    a2a_exchange(3, 4, 8)
                out_proj(3, ctxf_load(3), wo_sb, bob_sb)

                if dump:
                    nc.sync.dma_start(d_qT.ap(), qT[:])
                    nc.sync.dma_start(d_kT.ap(), kT[:])
                    nc.sync.dma_start(d_v.ap(), v_sb[:])
                    nc.sync.dma_start(d_e.ap(), e_sb[:, 1])

    nc.compile()
    return nc


_NC_CACHE = {}


def _get_nc(key):
    if key not in _NC_CACHE:
        kblocks, proj_fp8, out_fp8 = key
        _NC_CACHE[key] = build_nc(kblocks=kblocks, proj_fp8=proj_fp8,
                                  out_fp8=out_fp8)
    return _NC_CACHE[key]


# identical on every core -> uploaded once, replicated by XLA
_REPLICATED = {"wo", "bob", "ident"}

_RUNNER_CACHE = {}


def _make_runner(nc):
    import jax
    from jax.sharding import Mesh, NamedSharding, PartitionSpec as P
    from jax.experimental.shard_map import shard_map
    import concourse.bass2jax as b2j

    b2j.install_neuronx_cc_hook()
    in_names, out_names, out_avals = [], [], []
    for alloc in nc.m.functions[0].allocations:
        if not isinstance(alloc, mybir.MemoryLocationSet):
            continue
        name = alloc.memorylocations[0].name
        if alloc.kind == "ExternalInput":
            in_names.append(name)
        elif alloc.kind == "ExternalOutput":
            out_names.append(name)
            out_avals.append(
                jax.core.ShapedArray(
                    tuple(alloc.tensor_shape), mybir.dt.np(alloc.dtype)
                )
            )
    pid_name = nc.partition_id_tensor.name if nc.partition_id_tensor else None
    all_in_names = in_names + out_names

    def _body(*args):
        return tuple(
            b2j._bass_exec_p.bind(
                *args,
                out_avals=tuple(out_avals),
                in_names=tuple(all_in_names),
                out_names=tuple(out_names),
                lowering_input_output_aliases=(),
                sim_require_finite=True,
                sim_require_nnan=True,
                nc=nc,
            )
        )

    devices = jax.devices()[:N_CORES]
    mesh = Mesh(np.asarray(devices), ("core",))

    def spec_for(name):
        return P() if name in _REPLICATED else P("core")

    in_specs = tuple(spec_for(n) for n in in_names) + (P("core"),) * len(out_names)
    out_specs = (P("core"),) * len(out_names)
    fn = jax.jit(
        shard_map(_body, mesh=mesh, in_specs=in_specs, out_specs=out_specs,
                  check_rep=False),
        keep_unused=True,
    )
    sh_core = NamedSharding(mesh, P("core"))
    sh_repl = NamedSharding(mesh, P())
    zero_outs = [
        np.zeros((N_CORES * a.shape[0],) + tuple(a.shape[1:]), a.dtype)
        for a in out_avals
    ]
    upload_cache = {}

    def _put(name, arr, sh):
        import hashlib
        key = hashlib.blake2b(arr.tobytes(), digest_size=16).digest()
        hit = upload_cache.get(name)
        if hit is not None and hit[0] == key:
            return hit[1]
        buf = jax.device_put(arr, sh)
        upload_cache[name] = (key, buf)
        return buf

    def run(in_maps):
        args = []
        for name in in_names:
            if name == pid_name:
                cat = np.arange(N_CORES, dtype=np.uint32).reshape(N_CORES, 1)
                args.append(_put(name, cat, sh_core))
            elif name in _REPLICATED:
                args.append(_put(name, np.asarray(in_maps[0][name]), sh_repl))
            else:
                cat = np.concatenate(
                    [np.asarray(m[name]) for m in in_maps], axis=0
                )
                args.append(_put(name, cat, sh_core))
        for i, z in enumerate(zero_outs):
            args.append(_put(f"__zero{i}", z, sh_core))
        outs = fn(*args)
        jax.block_until_ready(outs)
        res = []
        for c in range(N_CORES):
            d = {}
            for i, name in enumerate(out_names):
                arr = np.asarray(outs[i])
                per = arr.shape[0] // N_CORES
                d[name] = arr[c * per:(c + 1) * per]
            res.append(d)
        return res

    return run


def _get_runner(key):
    if key not in _RUNNER_CACHE:
        _RUNNER_CACHE[key] = _make_runner(_get_nc(key))
    return _RUNNER_CACHE[key]


def _dr_pack(a, ncols):
    """[D, n] fp32 -> DoubleRow layout [NDP, 128, 2, n]: row d = 256g+128s+p."""
    return np.ascontiguousarray(a.reshape(NDP, 2, 128, ncols).swapaxes(1, 2))


def _x_chunks(a, W, proj_fp8):
    """[D, n] fp32 -> chunk-major x layout [n//W, 128, ., W]."""
    n = a.shape[1]
    nch = n // W
    if proj_fp8:
        # [g, s, p, c, w] -> [c, p, g, s, w]
        r = a.reshape(NDP, 2, 128, nch, W).transpose(3, 2, 0, 1, 4)
    else:
        r = a.reshape(NDT, 128, nch, W).transpose(2, 1, 0, 3)
    return np.ascontiguousarray(r)


def prepare_in_maps(kblocks, proj_fp8, out_fp8, query, key, value, mask,
                    Wq, bq, Wk, bk, Wv, bv, Wo, bo):
    import ml_dtypes
    bf16 = ml_dtypes.bfloat16
    f8 = ml_dtypes.float8_e4m3
    xnp = f8 if proj_fp8 else bf16
    cnp = f8 if out_fp8 else bf16
    SK = kblocks * 128
    m = np.asarray(mask).reshape(B, S)

    def wpack(a):  # [D, n] fp32 weight -> device layout
        if proj_fp8:
            return _dr_pack(a, a.shape[1]).astype(xnp)
        return np.ascontiguousarray(a.reshape(NDT, 128, a.shape[1])).astype(xnp)

    KW, QW, VW = min(256, SK), 512, min(512, SK)
    xq_b, xk_b, xv_b, maskb_b = [], [], [], []
    for b in range(B):
        idx = np.flatnonzero(m[b] == 0)
        n = len(idx)
        assert n <= SK, f"unmasked count {n} exceeds capacity {SK}"
        k_b = np.zeros((SK, D), np.float32)
        v_b = np.zeros((SK, D), np.float32)
        k_b[:n] = np.asarray(key, np.float32)[b][idx]
        v_b[:n] = np.asarray(value, np.float32)[b][idx]
        xq_b.append(_x_chunks(
            np.ascontiguousarray(np.asarray(query, np.float32)[b].T), QW,
            proj_fp8).astype(xnp))
        xk_b.append(_x_chunks(
            np.ascontiguousarray(k_b.T), KW, proj_fp8).astype(xnp))
        xv_b.append(_x_chunks(
            np.ascontiguousarray(v_b.T), VW, proj_fp8).astype(xnp))
        mb = np.full((kblocks, 128), -1e9, np.float32)
        mb.reshape(-1)[:n] = 0.0
        maskb_b.append(np.ascontiguousarray(mb.T))

    Wo_f = np.asarray(Wo, np.float32)
    if out_fp8:
        Wo_c = np.ascontiguousarray(
            Wo_f.reshape(GROUP, 2, 128, D).swapaxes(1, 2)).astype(cnp)
    else:
        Wo_c = np.ascontiguousarray(Wo_f.reshape(NDT, 128, D)).astype(cnp)
    bob = np.ascontiguousarray(
        np.broadcast_to(np.asarray(bo, np.float32), (128, D)))
    ident = np.eye(128, dtype=np.float32).astype(bf16)

    in_maps = []
    for c in range(N_CORES):
        b, r = c // GROUP, c % GROUP
        sl = slice(r * HL, (r + 1) * HL)
        bv_c = np.asarray(bv, np.float32)[sl]
        bvb = np.zeros((128, HPC * 65), np.float32)
        for h in range(HPC):
            bvb[:, h * 65:h * 65 + 64] = bv_c[h * 64:(h + 1) * 64]
            bvb[:, h * 65 + 64] = 1.0
        consts = np.concatenate([
            np.ascontiguousarray(
                np.asarray(bk, np.float32)[sl].reshape(2, 128).T),
            np.ascontiguousarray(
                np.asarray(bq, np.float32)[sl].reshape(2, 128).T),
            maskb_b[b],
            bvb,
        ], axis=1)
        in_maps.append({
            "xq": xq_b[b], "xk": xk_b[b], "xv": xv_b[b],
            "wq": wpack(np.ascontiguousarray(np.asarray(Wq, np.float32)[:, sl])),
            "wk": wpack(np.ascontiguousarray(np.asarray(Wk, np.float32)[:, sl])),
            "wv": wpack(np.ascontiguousarray(np.asarray(Wv, np.float32)[:, sl])),
            "wo": Wo_c,
            "consts": np.ascontiguousarray(consts),
            "bob": bob,
            "ident": ident,
        })
    return in_maps


def _pick_kblocks(mask):
    m = np.asarray(mask).reshape(B, S)
    maxn = int((m == 0).sum(axis=1).max())
    return min(S // 128, max(1, math.ceil(maxn / 128)))


PROJ_FP8 = False
OUT_FP8 = False


def kernel(**inputs) -> np.ndarray:
    kblocks = _pick_kblocks(inputs["mask"])
    in_maps = prepare_in_maps(kblocks, PROJ_FP8, OUT_FP8, **inputs)
    key = (kblocks, PROJ_FP8, OUT_FP8)
    try:
        run = _get_runner(key)
        results = run(in_maps)
    except Exception:
        res = bass_utils.run_bass_kernel_spmd(
            _get_nc(key), in_maps, core_ids=list(range(N_CORES))
        )
        results = res.results
    out = np.zeros((B, S, D), np.float32)
    for c in range(N_CORES):
        o = np.asarray(results[c]["out"], np.float32)  # [NQB, 128, D]
        for j in range(NQB):
            for beta in range(B):
                out[beta, j * 512 + c * 64:j * 512 + (c + 1) * 64] = \
                    o[j, beta * 64:(beta + 1) * 64]
    return out


# revision 3
# speedup vs baseline: 1.0451x; 1.0451x over previous
"""Multi-head attention (B=2, S=2048, D=1024, H=16) on 8 TRN2 NeuronCores, v2.

Sharding: 2-way batch data-parallel x 4-way head tensor-parallel.
Core c handles batch c//4 with heads [4r, 4r+4) where r = c%4.

Key ideas vs v1:
- bf16/fp8 activations+weights (less HBM traffic, same-or-better PE rate).
- key compaction without the +1 safety block (exact ceil(n/128) blocks).
- transposed PV (attn^T is exactly the exp output layout): ctx comes out
  [q, d] with full 128-partition outputs -> half the PE rows of v1's PV,
  and softmax normalization becomes a cheap per-partition scalar multiply.
- denominator via an all-ones column appended to each head's v tile.
- 1024-wide exp (2 heads x 512 q per Activation instruction, reading a
  double-buffered 2-bank PSUM tile).
- fp8(e4m3) DoubleRow projections: the host ships x and W pre-interleaved
  as [128, 2-slot, .] with contraction dim d = 256*g + 128*slot + p, so a
  256-deep contraction runs at 0.5 cycles/row with no on-device shuffle.
- optional fp8 output projection: ctxT's [dim-pair, token] layout is
  already DoubleRow-compatible (slot = head-pair index).
- AllToAll in 4 per-query-block rounds, each core taking a 128-token
  slice of every block, so output projection pipelines behind attention
  on every core symmetrically.
"""
import math

import numpy as np

import concourse.mybir as mybir
import concourse.tile as tile
from concourse import bacc, bass_utils

B, S, D, H = 2, 2048, 1024, 16
DEPTH = 64
N_CORES = 8
GROUP = 4              # cores per batch (tensor parallel over heads)
HPC = H // GROUP       # 4 heads per core
HL = HPC * DEPTH       # 256 local head dims
NDT = D // 128         # 8 contraction tiles of 128
NDP = NDT // 2         # 4 double-row contraction tiles of 256
NQB = S // 512         # 4 query blocks of 512 per batch

F32 = mybir.dt.float32
BF16 = mybir.dt.bfloat16
F8 = mybir.dt.float8e4
AFT = mybir.ActivationFunctionType
DR = mybir.MatmulPerfMode.DoubleRow


def build_nc(kblocks: int = 8, proj_fp8: bool = False, out_fp8: bool = False,
             collective: bool = True, num_devices: int = N_CORES,
             dump: bool = False, reps: int = 1):
    SK = kblocks * 128
    NKT = SK // 128
    xdt = F8 if proj_fp8 else BF16
    cdt = F8 if out_fp8 else BF16
    nc = bacc.Bacc(
        "TRN2", target_bir_lowering=False, debug=False, num_devices=num_devices
    )

    # ---- I/O (per-core slices prepared by the host) ----
    # x layouts are chunk-major so one chunk = one 2-dim DMA:
    #   fp8:  [nch, 128, NDP, 2, W]; row d = 256*g + 128*slot + p
    #   bf16: [nch, 128, NDT, W]
    # chunk widths: xk 256, xq 512, xv 512.
    KW, QW, VW = min(256, SK), 512, min(512, SK)
    NKC, NQC, NVC = SK // KW, S // QW, SK // VW
    xin = ([NDP, 2] if proj_fp8 else [NDT])
    xq = nc.dram_tensor("xq", [NQC, 128] + xin + [QW], xdt, kind="ExternalInput")
    xk = nc.dram_tensor("xk", [NKC, 128] + xin + [KW], xdt, kind="ExternalInput")
    xv = nc.dram_tensor("xv", [NVC, 128] + xin + [VW], xdt, kind="ExternalInput")
    wshape = ([NDP, 128, 2] if proj_fp8 else [NDT, 128])
    wq = nc.dram_tensor("wq", wshape + [HL], xdt, kind="ExternalInput")
    wk = nc.dram_tensor("wk", wshape + [HL], xdt, kind="ExternalInput")
    wv = nc.dram_tensor("wv", wshape + [HL], xdt, kind="ExternalInput")
    # wo: fp8 [GROUP*2, 128, 2, D] with row (i, slot, p) = ctx dim
    # 256i + 128*slot + p; bf16 [NDT, 128, D] plain.
    woshape = ([GROUP, 128, 2] if out_fp8 else [NDT, 128])
    wo = nc.dram_tensor("wo", woshape + [D], cdt, kind="ExternalInput")
    # consts packed in one tensor: bk | bq | maskb | bvb (f32 columns)
    NCC = 4 + kblocks + HPC * 65
    consts = nc.dram_tensor("consts", [128, NCC], F32, kind="ExternalInput")
    bob = nc.dram_tensor("bob", [128, D], F32, kind="ExternalInput")
    ident = nc.dram_tensor("ident", [128, 128], BF16, kind="ExternalInput")
    out = nc.dram_tensor("out", [NQB, 128, D], BF16, kind="ExternalOutput")

    if dump:
        d_qT = nc.dram_tensor("d_qT", [128, 2, S], BF16, kind="ExternalOutput")
        d_kT = nc.dram_tensor("d_kT", [128, 2, SK], BF16, kind="ExternalOutput")
        d_v = nc.dram_tensor("d_v", [128, NKT, HPC * 65], BF16,
                             kind="ExternalOutput")
        d_e = nc.dram_tensor("d_e", [128, kblocks, HPC * 512], BF16,
                             kind="ExternalOutput")

    # global 8-way AllToAll: chunk g = my 256 dims for tokens
    # [qb*512 + g*64, +64) of my batch; core c ends up with all 1024 dims of
    # BOTH batches' 64-token slice c.
    a2a_in = [nc.dram_tensor(f"a2a_in{j}", [N_CORES, 128, 2, 64], cdt)
              for j in range(NQB)]
    a2a_out = [nc.dram_tensor(f"a2a_out{j}", [N_CORES, 128, 2, 64], cdt)
               for j in range(NQB)]

    with tile.TileContext(nc) as tc:
        with (
            tc.tile_pool(name="w", bufs=1) as wp,
            tc.tile_pool(name="big", bufs=1) as bigp,
            tc.tile_pool(name="io", bufs=4) as iop,
            tc.tile_pool(name="ps", bufs=1, space="PSUM") as ps,
        ):
            # ---- constants ----
            wsl = [NDP, 2] if proj_fp8 else [NDT]
            wq_sb = wp.tile([128] + wsl + [HL], xdt)
            wk_sb = wp.tile([128] + wsl + [HL], xdt)
            wv_sb = wp.tile([128] + wsl + [HL], xdt)
            wre = "g p s m -> p g s m" if proj_fp8 else "g p m -> p g m"
            const_sb = wp.tile([128, NCC], F32)
            bk_sb = const_sb[:, 0:2]
            bq_sb = const_sb[:, 2:4]
            maskb_sb = const_sb[:, 4:4 + kblocks]
            bvb_sb = const_sb[:, 4 + kblocks:4 + kblocks + HPC * 65]
            ident_sb = wp.tile([128, 128], BF16)
            # K-path constants first so the first exp fires as early as
            # possible; wq/wv/remaining consts are queued mid-schedule.
            nc.sync.dma_start(const_sb[:], consts.ap())

            # ---- persistent activations (one tile per input chunk) ----
            xsl = [NDP, 2] if proj_fp8 else [NDT]
            xq_sb = [bigp.tile([128] + xsl + [QW], xdt, name=f"xq{i}")
                     for i in range(NQC)]
            xk_sb = [bigp.tile([128] + xsl + [KW], xdt, name=f"xk{i}")
                     for i in range(NKC)]
            xv_sb = [bigp.tile([128] + xsl + [VW], xdt, name=f"xv{i}")
                     for i in range(NVC)]
            qT = bigp.tile([128, 2, S], BF16)    # local q dim j*128+p
            kT = bigp.tile([128, 2, SK], BF16)
            v_sb = bigp.tile([128, NKT, HPC * 65], BF16)
            e_sb = bigp.tile([128, 2, kblocks, HPC * 512], BF16)

            # ones columns (denominator trick): v_sb[:, :, h*65+64] = 1
            nc.vector.memset(
                v_sb[:].rearrange("p t (h c) -> p t h c", h=HPC)[:, :, :, 64], 1.0
            )

            # warm the Exp activation table off the critical path
            warm = iop.tile([1, 1], F32, tag="warm", bufs=1, name="warm")
            nc.scalar.activation(warm[:], warm[:], AFT.Exp)

            # ---------- emission helpers ----------
            def load_chunk(x_sb, x_dram, c):
                nc.sync.dma_start(x_sb[c][:], x_dram.ap()[c])

            NG = NDP if proj_fp8 else NDT

            def qk_proj(x_t, w_sb, b_sb, dst, cc, w, js=(0, 1)):
                """project chunk tile x_t -> dst[:, j, cc:cc+w]."""
                for j in js:
                    ps_p = ps.tile([128, 512], F32, tag="mm", bufs=2, name="ps_p")
                    for g in range(NG):
                        if proj_fp8:
                            nc.tensor.matmul(
                                ps_p[:, 0:w],
                                w_sb[:, g, :, j * 128:(j + 1) * 128],
                                x_t[:, g, :, 0:w],
                                perf_mode=DR,
                                start=(g == 0), stop=(g == NG - 1),
                            )
                        else:
                            nc.tensor.matmul(
                                ps_p[:, 0:w], w_sb[:, g, j * 128:(j + 1) * 128],
                                x_t[:, g, 0:w],
                                start=(g == 0), stop=(g == NG - 1),
                            )
                    nc.vector.tensor_scalar_add(
                        dst[:, j, cc:cc + w], ps_p[:, 0:w], b_sb[:, j:j + 1]
                    )

            def v_proj(kt):
                """value projection for key tile kt -> v_sb[:, kt, :]."""
                x_t = xv_sb[(kt * 128) // VW]
                c0 = (kt * 128) % VW
                ps_v = ps.tile([128, 512], F32, tag="mm", bufs=2, name="ps_v")
                for g in range(NG):
                    if proj_fp8:
                        nc.tensor.matmul(
                            ps_v[:, 0:HL],
                            x_t[:, g, :, c0:c0 + 128],
                            wv_sb[:, g, :, :],
                            perf_mode=DR,
                            start=(g == 0), stop=(g == NG - 1),
                        )
                    else:
                        nc.tensor.matmul(
                            ps_v[:, 0:HL], x_t[:, g, c0:c0 + 128],
                            wv_sb[:, g, :],
                            start=(g == 0), stop=(g == NG - 1),
                        )
                pv4 = ps_v[:, 0:HL].rearrange("p (h c) -> p h c", h=HPC)
                vb4 = bvb_sb.rearrange("p (h c) -> p h c", h=HPC)[:, :, 0:64]
                dst = v_sb[:, kt, :].rearrange("p (h c) -> p h c", h=HPC)[:, :, 0:64]
                nc.vector.tensor_add(dst, pv4, vb4)

            def qk_exp(qb, kb):
                """logits + exp, two heads per Activation instruction."""
                for jp in range(2):  # head pair (jp*2, jp*2+1)
                    lg = ps.tile([128, 1024], F32, tag="lg", bufs=2, name="lg")
                    for hh in range(2):
                        h = jp * 2 + hh
                        p0 = (h % 2) * 64
                        nc.tensor.matmul(
                            lg[:, hh * 512:(hh + 1) * 512],
                            kT[p0:p0 + 64, h // 2, kb * 128:(kb + 1) * 128],
                            qT[p0:p0 + 64, h // 2, qb * 512:(qb + 1) * 512],
                            start=True, stop=True,
                        )
                    nc.scalar.activation(
                        e_sb[:, qb % 2, kb, jp * 1024:(jp + 1) * 1024], lg[:],
                        AFT.Exp, bias=maskb_sb[:, kb:kb + 1], scale=0.125,
                    )

            cn_tiles = {}

            def pv_mm(qb, qq):
                """PV matmuls + normalize for one 128-query chunk."""
                cx = ps.tile([128, HPC * 65], F32, tag="cx", bufs=2, name="cx")
                for h in range(HPC):
                    for kb in range(kblocks):
                        nc.tensor.matmul(
                            cx[:, h * 65:(h + 1) * 65],
                            e_sb[:, qb % 2, kb,
                                 h * 512 + qq * 128:h * 512 + (qq + 1) * 128],
                            v_sb[:, kb, h * 65:(h + 1) * 65],
                            start=(kb == 0), stop=(kb == kblocks - 1),
                        )
                rcp = iop.tile([128, 4], F32, tag="rcp", bufs=3, name="rcp")
                dens = cx[:].rearrange("p (h c) -> p h c", h=HPC)[:, :, 64]
                nc.vector.reciprocal(rcp[:], dens)
                for hp in range(2):
                    cn = iop.tile([128, 128], BF16, tag="cn", bufs=8, name="cn")
                    for hh in range(2):
                        h = hp * 2 + hh
                        if qb == 3:
                            # Act is idle after the last exp
                            nc.scalar.activation(
                                cn[:, hh * 64:(hh + 1) * 64],
                                cx[:, h * 65:h * 65 + 64], AFT.Copy,
                                scale=rcp[:, h:h + 1],
                            )
                        else:
                            nc.vector.tensor_scalar_mul(
                                cn[:, hh * 64:(hh + 1) * 64],
                                cx[:, h * 65:h * 65 + 64], rcp[:, h:h + 1],
                            )
                    cn_tiles[(qq, hp)] = cn

            def pv_tr(qb, qqs):
                """transpose + copy + a2a staging for query chunks."""
                for qq in qqs:
                    ctq = iop.tile([128, 2, 128], cdt, tag="ctq", bufs=4,
                                   name="ctq")
                    for hp in range(2):
                        cn = cn_tiles.pop((qq, hp))
                        # transpose via PE into a bitcast view of an mm tile
                        tp = ps.tile([128, 512], F32, tag="mm", bufs=2, name="tp")
                        tpv = tp[:, 0:64].bitcast(BF16)
                        nc.tensor.transpose(tpv, cn[:], ident_sb[:])
                        if qb == 3:
                            # Act engine is idle after the last exp; keep the
                            # tail's mul->transpose->copy chain off the DVE
                            nc.scalar.copy(ctq[:, hp], tpv)
                        else:
                            nc.vector.tensor_copy(ctq[:, hp], tpv)
                    # stage as two 64-token chunks for the exchange
                    for s in range(2):
                        nc.sync.dma_start(
                            a2a_in[qb].ap()[2 * qq + s],
                            ctq[:, :, s * 64:(s + 1) * 64],
                        )

            def a2a_exchange(j, g0=0, g1=N_CORES):
                if collective:
                    assert (g0, g1) == (0, N_CORES)
                    nc.gpsimd.collective_compute(
                        "AllToAll",
                        mybir.AluOpType.bypass,
                        replica_groups=[list(range(N_CORES))],
                        ins=[a2a_in[j].ap().opt()],
                        outs=[a2a_out[j].ap().opt()],
                    )
                else:
                    nc.gpsimd.dma_start(
                        a2a_out[j].ap()[g0:g1], a2a_in[j].ap()[g0:g1]
                    )

            def ctxf_load(j, halves=(0, 1), ctxf=None):
                # free layout (g, hp, b*64+t): each (g, hp) slice is a valid
                # single-free-dim stationary operand covering both batches
                if ctxf is None:
                    ctxf = iop.tile([128, GROUP, 2, 128], cdt, tag="ctxf",
                                    bufs=2, name="ctxf")
                for b in halves:
                    for hp in range(2):
                        nc.sync.dma_start(
                            ctxf[:, :, hp, b * 64:(b + 1) * 64],
                            a2a_out[j].ap()[b * 4:(b + 1) * 4, :, hp].rearrange(
                                "g p t -> p g t"),
                        )
                return ctxf

            def out_proj_bb(j, ctxf, wo_sb, bob_sb):
                """out proj split by batch: batch-0 matmuls only need the
                first four exchanged chunks (ctxf cols 0:64)."""
                o_sb = iop.tile([128, D], BF16, tag="osb", bufs=2, name="o_sb")
                for half in range(2):
                    for b in range(2):
                        ps_o = ps.tile([128, 512], F32, tag="mm", bufs=2,
                                       name="ps_o")
                        n = 0
                        for g in range(GROUP):
                            for hp in range(2):
                                nc.tensor.matmul(
                                    ps_o[0:64, :],
                                    ctxf[:, g, hp, b * 64:(b + 1) * 64],
                                    wo_sb[:, g * 2 + hp,
                                          half * 512:(half + 1) * 512],
                                    start=(n == 0), stop=(n == 2 * GROUP - 1),
                                )
                                n += 1
                        nc.vector.tensor_add(
                            o_sb[b * 64:(b + 1) * 64,
                                 half * 512:(half + 1) * 512],
                            ps_o[0:64, :],
                            bob_sb[0:64, half * 512:(half + 1) * 512],
                        )
                    nc.sync.dma_start(
                        out.ap()[j, :, half * 512:(half + 1) * 512],
                        o_sb[:, half * 512:(half + 1) * 512],
                    )

            def out_proj(j, ctxf, wo_sb, bob_sb, halves=(0, 1), o_sb=None):
                # rows 0:64 = my 64-token slice of batch 0, rows 64:128 batch 1
                if o_sb is None:
                    o_sb = iop.tile([128, D], BF16, tag="osb", bufs=2,
                                    name="o_sb")
                for half in halves:
                    ps_o = ps.tile([128, 512], F32, tag="mm", bufs=2,
                                   name="ps_o")
                    n = 0
                    for g in range(GROUP):
                        for hp in range(2):
                            nc.tensor.matmul(
                                ps_o[:], ctxf[:, g, hp, :],
                                wo_sb[:, g * 2 + hp,
                                      half * 512:(half + 1) * 512],
                                start=(n == 0), stop=(n == 2 * GROUP - 1),
                            )
                            n += 1
                    nc.vector.tensor_add(
                        o_sb[:, half * 512:(half + 1) * 512], ps_o[:],
                        bob_sb[:, half * 512:(half + 1) * 512],
                    )
                    nc.sync.dma_start(
                        out.ap()[j, :, half * 512:(half + 1) * 512],
                        o_sb[:, half * 512:(half + 1) * 512],
                    )
                return o_sb

            # ---------- schedule ----------
            if out_fp8:
                wo_sb = wp.tile([128, GROUP, 2, D], F8)
            else:
                wo_sb = wp.tile([128, NDT, D], BF16)
            bob_sb = wp.tile([128, D], F32)

            for rep in range(reps):
                # ---- round 0: all projections + exp(0) stream.
                # first chunks and weights arrive in interleaved g-halves so
                # the first projection matmuls start ~4us earlier ----
                H0 = NG // 2
                if rep == 0:
                    nc.sync.dma_start(wk_sb[:, 0:H0], wk.ap()[0:H0].rearrange(wre))
                nc.sync.dma_start(xk_sb[0][:, 0:H0], xk.ap()[0, :, 0:H0])
                if rep == 0:
                    nc.sync.dma_start(wq_sb[:, 0:H0], wq.ap()[0:H0].rearrange(wre))
                nc.sync.dma_start(xq_sb[0][:, 0:H0], xq.ap()[0, :, 0:H0])
                if rep == 0:
                    nc.sync.dma_start(wk_sb[:, H0:], wk.ap()[H0:].rearrange(wre))
                nc.sync.dma_start(xk_sb[0][:, H0:], xk.ap()[0, :, H0:])
                if rep == 0:
                    nc.sync.dma_start(wq_sb[:, H0:], wq.ap()[H0:].rearrange(wre))
                nc.sync.dma_start(xq_sb[0][:, H0:], xq.ap()[0, :, H0:])
                qk_proj(xk_sb[0], wk_sb, bk_sb, kT, 0, KW)
                qk_proj(xq_sb[0], wq_sb, bq_sb, qT, 0, 512)
                for kb in range(min(2, kblocks)):
                    qk_exp(0, kb)
                for c in range(1, NKC):
                    load_chunk(xk_sb, xk, c)
                if rep == 0:
                    nc.sync.dma_start(ident_sb[:], ident.ap())
                for c in range(1, NQC):
                    load_chunk(xq_sb, xq, c)
                for c in range(1, NKC):
                    qk_proj(xk_sb[c], wk_sb, bk_sb, kT, c * KW, KW)
                for kb in range(2, min(4, kblocks)):
                    qk_exp(0, kb)
                qk_proj(xq_sb[1], wq_sb, bq_sb, qT, 512, 512)
                for kb in range(4, kblocks):
                    qk_exp(0, kb)
                if rep == 0:
                    nc.sync.dma_start(wv_sb[:], wv.ap().rearrange(wre))
                for c in range(NVC):
                    load_chunk(xv_sb, xv, c)
                for kt in range(min(6, NKT)):
                    v_proj(kt)
                if rep == 0:
                    nc.sync.dma_start(
                        wo_sb[:],
                        wo.ap().rearrange(
                            "g p s m -> p g s m" if out_fp8 else "g p m -> p g m"
                        ),
                    )
                    nc.sync.dma_start(bob_sb[:], bob.ap())

                # ---- rounds 1-3: exp(r) stream hosting round r-1's
                # pv/exchange and round r-2's output projection ----
                ctxf_t = {}
                ob_t = {}
                for r in range(1, NQB):
                    qk_exp(r, 0)
                    if r == 1 and NKT > 6:
                        v_proj(6)
                    qk_exp(r, 1)
                    if r == 1 and NKT > 7:
                        v_proj(7)
                    if r >= 2:
                        cf = ctxf_t[r - 2]
                        ob_t[r - 2] = out_proj(r - 2, cf, wo_sb, bob_sb, (0,))
                    qk_exp(r, 2)
                    if r < NQB - 1:
                        qk_proj(xq_sb[r + 1], wq_sb, bq_sb, qT,
                                (r + 1) * 512, 512, js=(0,))
                    if r == 3:
                        out_proj(0, ctxf_t.pop(0), wo_sb, bob_sb, (1,),
                                 ob_t.pop(0))
                    pv_mm(r - 1, 0)
                    qk_exp(r, 3)
                    if r < NQB - 1:
                        qk_proj(xq_sb[r + 1], wq_sb, bq_sb, qT,
                                (r + 1) * 512, 512, js=(1,))
                    pv_mm(r - 1, 1)
                    pv_tr(r - 1, (0,))
                    qk_exp(r, 4)
                    pv_mm(r - 1, 2)
                    pv_tr(r - 1, (1,))
                    qk_exp(r, 5)
                    pv_mm(r - 1, 3)
                    pv_tr(r - 1, (2,))
                    qk_exp(r, 6)
                    pv_tr(r - 1, (3,))
                    if not collective:
                        a2a_exchange(r - 1, 0, 4)
                    qk_exp(r, 7)
                    if collective:
                        a2a_exchange(r - 1)
                    else:
                        a2a_exchange(r - 1, 4, 8)
                    ctxf_t[r - 1] = ctxf_load(r - 1)

                # ---- tail: last block's pv, then the two outputs ----
                for qq in range(4):
                    pv_mm(3, qq)
                    pv_tr(3, (qq,))
                    if qq == 1 and not collective:
                        a2a_exchange(3, 0, 4)
                out_proj(1, ctxf_t.pop(1), wo_sb, bob_sb, (1,), ob_t.pop(1))
                out_proj(2, ctxf_t.pop(2), wo_sb, bob_sb)
                if collective:
                    a2a_exchange(3)
                else:
                    a2a_exchange(3, 4, 8)
                out_proj(3, ctxf_load(3), wo_sb, bob_sb)

                if dump:
                    nc.sync.dma_start(d_qT.ap(), qT[:])
                    nc.sync.dma_start(d_kT.ap(), kT[:])
                    nc.sync.dma_start(d_v.ap(), v_sb[:])
                    nc.sync.dma_start(d_e.ap(), e_sb[:, 1])

    nc.compile()
    return nc


_NC_CACHE = {}


def _get_nc(key):
    if key not in _NC_CACHE:
        kblocks, proj_fp8, out_fp8 = key
        _NC_CACHE[key] = build_nc(kblocks=kblocks, proj_fp8=proj_fp8,
                                  out_fp8=out_fp8)
    return _NC_CACHE[key]


# identical on every core -> uploaded once, replicated by XLA
_REPLICATED = {"wo", "bob", "ident"}

_RUNNER_CACHE = {}


def _make_runner(nc):
    import jax
    from jax.sharding import Mesh, NamedSharding, PartitionSpec as P
    from jax.experimental.shard_map import shard_map
    import concourse.bass2jax as b2j

    b2j.install_neuronx_cc_hook()
    in_names, out_names, out_avals = [], [], []
    for alloc in nc.m.functions[0].allocations:
        if not isinstance(alloc, mybir.MemoryLocationSet):
            continue
        name = alloc.memorylocations[0].name
        if alloc.kind == "ExternalInput":
            in_names.append(name)
        elif alloc.kind == "ExternalOutput":
            out_names.append(name)
            out_avals.append(
                jax.core.ShapedArray(
                    tuple(alloc.tensor_shape), mybir.dt.np(alloc.dtype)
                )
            )
    pid_name = nc.partition_id_tensor.name if nc.partition_id_tensor else None
    all_in_names = in_names + out_names

    def _body(*args):
        return tuple(
            b2j._bass_exec_p.bind(
                *args,
                out_avals=tuple(out_avals),
                in_names=tuple(all_in_names),
                out_names=tuple(out_names),
                lowering_input_output_aliases=(),
                sim_require_finite=True,
                sim_require_nnan=True,
                nc=nc,
            )
        )

    devices = jax.devices()[:N_CORES]
    mesh = Mesh(np.asarray(devices), ("core",))

    def spec_for(name):
        return P() if name in _REPLICATED else P("core")

    in_specs = tuple(spec_for(n) for n in in_names) + (P("core"),) * len(out_names)
    out_specs = (P("core"),) * len(out_names)
    fn = jax.jit(
        shard_map(_body, mesh=mesh, in_specs=in_specs, out_specs=out_specs,
                  check_rep=False),
        keep_unused=True,
    )
    sh_core = NamedSharding(mesh, P("core"))
    sh_repl = NamedSharding(mesh, P())
    zero_outs = [
        np.zeros((N_CORES * a.shape[0],) + tuple(a.shape[1:]), a.dtype)
        for a in out_avals
    ]
    upload_cache = {}

    def _put(name, arr, sh):
        import hashlib
        key = hashlib.blake2b(arr.tobytes(), digest_size=16).digest()
        hit = upload_cache.get(name)
        if hit is not None and hit[0] == key:
            return hit[1]
        buf = jax.device_put(arr, sh)
        upload_cache[name] = (key, buf)
        return buf

    def run(in_maps):
        args = []
        for name in in_names:
            if name == pid_name:
                cat = np.arange(N_CORES, dtype=np.uint32).reshape(N_CORES, 1)
                args.append(_put(name, cat, sh_core))
            elif name in _REPLICATED:
                args.append(_put(name, np.asarray(in_maps[0][name]), sh_repl))
            else:
                cat = np.concatenate(
                    [np.asarray(m[name]) for m in in_maps], axis=0
                )
                args.append(_put(name, cat, sh_core))
        for i, z in enumerate(zero_outs):
            args.append(_put(f"__zero{i}", z, sh_core))
        outs = fn(*args)
        jax.block_until_ready(outs)
        res = []
        for c in range(N_CORES):
            d = {}
            for i, name in enumerate(out_names):
                arr = np.asarray(outs[i])
                per = arr.shape[0] // N_CORES
                d[name] = arr[c * per:(c + 1) * per]
            res.append(d)
        return res

    return run


def _get_runner(key):
    if key not in _RUNNER_CACHE:
        _RUNNER_CACHE[key] = _make_runner(_get_nc(key))
    return _RUNNER_CACHE[key]


def _dr_pack(a, ncols):
    """[D, n] fp32 -> DoubleRow layout [NDP, 128, 2, n]: row d = 256g+128s+p."""
    return np.ascontiguousarray(a.reshape(NDP, 2, 128, ncols).swapaxes(1, 2))


def _x_chunks(a, W, proj_fp8):
    """[D, n] fp32 -> chunk-major x layout [n//W, 128, ., W]."""
    n = a.shape[1]
    nch = n // W
    if proj_fp8:
        # [g, s, p, c, w] -> [c, p, g, s, w]
        r = a.reshape(NDP, 2, 128, nch, W).transpose(3, 2, 0, 1, 4)
    else:
        r = a.reshape(NDT, 128, nch, W).transpose(2, 1, 0, 3)
    return np.ascontiguousarray(r)


def prepare_in_maps(kblocks, proj_fp8, out_fp8, query, key, value, mask,
                    Wq, bq, Wk, bk, Wv, bv, Wo, bo):
    import ml_dtypes
    bf16 = ml_dtypes.bfloat16
    f8 = ml_dtypes.float8_e4m3
    xnp = f8 if proj_fp8 else bf16
    cnp = f8 if out_fp8 else bf16
    SK = kblocks * 128
    m = np.asarray(mask).reshape(B, S)

    def wpack(a):  # [D, n] fp32 weight -> device layout
        if proj_fp8:
            return _dr_pack(a, a.shape[1]).astype(xnp)
        return np.ascontiguousarray(a.reshape(NDT, 128, a.shape[1])).astype(xnp)

    KW, QW, VW = min(256, SK), 512, min(512, SK)
    xq_b, xk_b, xv_b, maskb_b = [], [], [], []
    for b in range(B):
        idx = np.flatnonzero(m[b] == 0)
        n = len(idx)
        assert n <= SK, f"unmasked count {n} exceeds capacity {SK}"
        k_b = np.zeros((SK, D), np.float32)
        v_b = np.zeros((SK, D), np.float32)
        k_b[:n] = np.asarray(key, np.float32)[b][idx]
        v_b[:n] = np.asarray(value, np.float32)[b][idx]
        xq_b.append(_x_chunks(
            np.ascontiguousarray(np.asarray(query, np.float32)[b].T), QW,
            proj_fp8).astype(xnp))
        xk_b.append(_x_chunks(
            np.ascontiguousarray(k_b.T), KW, proj_fp8).astype(xnp))
        xv_b.append(_x_chunks(
            np.ascontiguousarray(v_b.T), VW, proj_fp8).astype(xnp))
        mb = np.full((kblocks, 128), -1e9, np.float32)
        mb.reshape(-1)[:n] = 0.0
        maskb_b.append(np.ascontiguousarray(mb.T))

    Wo_f = np.asarray(Wo, np.float32)
    if out_fp8:
        Wo_c = np.ascontiguousarray(
            Wo_f.reshape(GROUP, 2, 128, D).swapaxes(1, 2)).astype(cnp)
    else:
        Wo_c = np.ascontiguousarray(Wo_f.reshape(NDT, 128, D)).astype(cnp)
    bob = np.ascontiguousarray(
        np.broadcast_to(np.asarray(bo, np.float32), (128, D)))
    ident = np.eye(128, dtype=np.float32).astype(bf16)

    in_maps = []
    for c in range(N_CORES):
        b, r = c // GROUP, c % GROUP
        sl = slice(r * HL, (r + 1) * HL)
        bv_c = np.asarray(bv, np.float32)[sl]
        bvb = np.zeros((128, HPC * 65), np.float32)
        for h in range(HPC):
            bvb[:, h * 65:h * 65 + 64] = bv_c[h * 64:(h + 1) * 64]
            bvb[:, h * 65 + 64] = 1.0
        consts = np.concatenate([
            np.ascontiguousarray(
                np.asarray(bk, np.float32)[sl].reshape(2, 128).T),
            np.ascontiguousarray(
                np.asarray(bq, np.float32)[sl].reshape(2, 128).T),
            maskb_b[b],
            bvb,
        ], axis=1)
        in_maps.append({
            "xq": xq_b[b], "xk": xk_b[b], "xv": xv_b[b],
            "wq": wpack(np.ascontiguousarray(np.asarray(Wq, np.float32)[:, sl])),
            "wk": wpack(np.ascontiguousarray(np.asarray(Wk, np.float32)[:, sl])),
            "wv": wpack(np.ascontiguousarray(np.asarray(Wv, np.float32)[:, sl])),
            "wo": Wo_c,
            "consts": np.ascontiguousarray(consts),
            "bob": bob,
            "ident": ident,
        })
    return in_maps


def _pick_kblocks(mask):
    m = np.asarray(mask).reshape(B, S)
    maxn = int((m == 0).sum(axis=1).max())
    return min(S // 128, max(1, math.ceil(maxn / 128)))


PROJ_FP8 = False
OUT_FP8 = False


def kernel(**inputs) -> np.ndarray:
    kblocks = _pick_kblocks(inputs["mask"])
    in_maps = prepare_in_maps(kblocks, PROJ_FP8, OUT_FP8, **inputs)
    key = (kblocks, PROJ_FP8, OUT_FP8)
    try:
        run = _get_runner(key)
        results = run(in_maps)
    except Exception:
        res = bass_utils.run_bass_kernel_spmd(
            _get_nc(key), in_maps, core_ids=list(range(N_CORES))
        )
        results = res.results
    out = np.zeros((B, S, D), np.float32)
    for c in range(N_CORES):
        o = np.asarray(results[c]["out"], np.float32)  # [NQB, 128, D]
        for j in range(NQB):
            for beta in range(B):
                out[beta, j * 512 + c * 64:j * 512 + (c + 1) * 64] = \
                    o[j, beta * 64:(beta + 1) * 64]
    return out


# revision 4
# speedup vs baseline: 1.0541x; 1.0085x over previous
"""Multi-head attention (B=2, S=2048, D=1024, H=16) on 8 TRN2 NeuronCores, v2.

Sharding: 2-way batch data-parallel x 4-way head tensor-parallel.
Core c handles batch c//4 with heads [4r, 4r+4) where r = c%4.

Key ideas vs v1:
- bf16/fp8 activations+weights (less HBM traffic, same-or-better PE rate).
- key compaction without the +1 safety block (exact ceil(n/128) blocks).
- transposed PV (attn^T is exactly the exp output layout): ctx comes out
  [q, d] with full 128-partition outputs -> half the PE rows of v1's PV,
  and softmax normalization becomes a cheap per-partition scalar multiply.
- denominator via an all-ones column appended to each head's v tile.
- 1024-wide exp (2 heads x 512 q per Activation instruction, reading a
  double-buffered 2-bank PSUM tile).
- fp8(e4m3) DoubleRow projections: the host ships x and W pre-interleaved
  as [128, 2-slot, .] with contraction dim d = 256*g + 128*slot + p, so a
  256-deep contraction runs at 0.5 cycles/row with no on-device shuffle.
- optional fp8 output projection: ctxT's [dim-pair, token] layout is
  already DoubleRow-compatible (slot = head-pair index).
- AllToAll in 4 per-query-block rounds, each core taking a 128-token
  slice of every block, so output projection pipelines behind attention
  on every core symmetrically.
"""
import math

import numpy as np

import concourse.mybir as mybir
import concourse.tile as tile
from concourse import bacc, bass_utils

B, S, D, H = 2, 2048, 1024, 16
DEPTH = 64
N_CORES = 8
GROUP = 4              # cores per batch (tensor parallel over heads)
HPC = H // GROUP       # 4 heads per core
HL = HPC * DEPTH       # 256 local head dims
NDT = D // 128         # 8 contraction tiles of 128
NDP = NDT // 2         # 4 double-row contraction tiles of 256
NQB = S // 512         # 4 query blocks of 512 per batch

F32 = mybir.dt.float32
BF16 = mybir.dt.bfloat16
F8 = mybir.dt.float8e4
AFT = mybir.ActivationFunctionType
DR = mybir.MatmulPerfMode.DoubleRow


def build_nc(kblocks: int = 8, proj_fp8: bool = False, out_fp8: bool = False,
             collective: bool = True, num_devices: int = N_CORES,
             dump: bool = False, reps: int = 1):
    SK = kblocks * 128
    NKT = SK // 128
    xdt = F8 if proj_fp8 else BF16
    cdt = F8 if out_fp8 else BF16
    nc = bacc.Bacc(
        "TRN2", target_bir_lowering=False, debug=False, num_devices=num_devices
    )

    # ---- I/O (per-core slices prepared by the host) ----
    # x layouts are chunk-major so one chunk = one 2-dim DMA:
    #   fp8:  [nch, 128, NDP, 2, W]; row d = 256*g + 128*slot + p
    #   bf16: [nch, 128, NDT, W]
    # chunk widths: xk 256, xq 512, xv 512.
    KW, QW, VW = min(256, SK), 512, min(512, SK)
    NKC, NQC, NVC = SK // KW, S // QW, SK // VW
    xin = ([NDP, 2] if proj_fp8 else [NDT])
    xq = nc.dram_tensor("xq", [NQC, 128] + xin + [QW], xdt, kind="ExternalInput")
    xk = nc.dram_tensor("xk", [NKC, 128] + xin + [KW], xdt, kind="ExternalInput")
    xv = nc.dram_tensor("xv", [NVC, 128] + xin + [VW], xdt, kind="ExternalInput")
    wshape = ([NDP, 128, 2] if proj_fp8 else [NDT, 128])
    wq = nc.dram_tensor("wq", wshape + [HL], xdt, kind="ExternalInput")
    wk = nc.dram_tensor("wk", wshape + [HL], xdt, kind="ExternalInput")
    wv = nc.dram_tensor("wv", wshape + [HL], xdt, kind="ExternalInput")
    # wo: fp8 [GROUP*2, 128, 2, D] with row (i, slot, p) = ctx dim
    # 256i + 128*slot + p; bf16 [NDT, 128, D] plain.
    woshape = ([GROUP, 128, 2] if out_fp8 else [NDT, 128])
    wo = nc.dram_tensor("wo", woshape + [D], cdt, kind="ExternalInput")
    # consts packed in one tensor: bk | bq | maskb | bvb (f32 columns)
    NCC = 4 + kblocks + HPC * 65
    consts = nc.dram_tensor("consts", [128, NCC], F32, kind="ExternalInput")
    bob = nc.dram_tensor("bob", [128, D], F32, kind="ExternalInput")
    ident = nc.dram_tensor("ident", [128, 128], BF16, kind="ExternalInput")
    out = nc.dram_tensor("out", [NQB, 128, D], BF16, kind="ExternalOutput")

    if dump:
        d_qT = nc.dram_tensor("d_qT", [128, 2, S], BF16, kind="ExternalOutput")
        d_kT = nc.dram_tensor("d_kT", [128, 2, SK], BF16, kind="ExternalOutput")
        d_v = nc.dram_tensor("d_v", [128, NKT, HPC * 65], BF16,
                             kind="ExternalOutput")
        d_e = nc.dram_tensor("d_e", [128, kblocks, HPC * 512], BF16,
                             kind="ExternalOutput")

    # global 8-way AllToAll: chunk g = my 256 dims for tokens
    # [qb*512 + g*64, +64) of my batch; core c ends up with all 1024 dims of
    # BOTH batches' 64-token slice c.
    a2a_in = [nc.dram_tensor(f"a2a_in{j}", [N_CORES, 128, 2, 64], cdt)
              for j in range(NQB)]
    a2a_out = [nc.dram_tensor(f"a2a_out{j}", [N_CORES, 128, 2, 64], cdt)
               for j in range(NQB)]

    with tile.TileContext(nc) as tc:
        with (
            tc.tile_pool(name="w", bufs=1) as wp,
            tc.tile_pool(name="big", bufs=1) as bigp,
            tc.tile_pool(name="io", bufs=4) as iop,
            tc.tile_pool(name="ps", bufs=1, space="PSUM") as ps,
        ):
            # ---- constants ----
            wsl = [NDP, 2] if proj_fp8 else [NDT]
            wq_sb = wp.tile([128] + wsl + [HL], xdt)
            wk_sb = wp.tile([128] + wsl + [HL], xdt)
            wv_sb = wp.tile([128] + wsl + [HL], xdt)
            wre = "g p s m -> p g s m" if proj_fp8 else "g p m -> p g m"
            const_sb = wp.tile([128, NCC], F32)
            bk_sb = const_sb[:, 0:2]
            bq_sb = const_sb[:, 2:4]
            maskb_sb = const_sb[:, 4:4 + kblocks]
            bvb_sb = const_sb[:, 4 + kblocks:4 + kblocks + HPC * 65]
            ident_sb = wp.tile([128, 128], BF16)

            # ---- persistent activations (one tile per input chunk) ----
            xsl = [NDP, 2] if proj_fp8 else [NDT]
            xq_sb = [bigp.tile([128] + xsl + [QW], xdt, name=f"xq{i}")
                     for i in range(NQC)]
            xk_sb = [bigp.tile([128] + xsl + [KW], xdt, name=f"xk{i}")
                     for i in range(NKC)]
            xv_sb = [bigp.tile([128] + xsl + [VW], xdt, name=f"xv{i}")
                     for i in range(NVC)]
            qT = bigp.tile([128, 2, S], BF16)    # local q dim j*128+p
            kT = bigp.tile([128, 2, SK], BF16)
            v_sb = bigp.tile([128, NKT, HPC * 65], BF16)
            e_sb = bigp.tile([128, 2, kblocks, HPC * 512], BF16)

            # ones columns (denominator trick): v_sb[:, :, h*65+64] = 1
            nc.vector.memset(
                v_sb[:].rearrange("p t (h c) -> p t h c", h=HPC)[:, :, :, 64], 1.0
            )

            # warm the Exp activation table off the critical path
            warm = iop.tile([1, 1], F32, tag="warm", bufs=1, name="warm")
            nc.scalar.activation(warm[:], warm[:], AFT.Exp)

            # ---------- emission helpers ----------
            def load_chunk(x_sb, x_dram, c):
                nc.sync.dma_start(x_sb[c][:], x_dram.ap()[c])

            NG = NDP if proj_fp8 else NDT

            def qk_proj(x_t, w_sb, b_sb, dst, cc, w, js=(0, 1)):
                """project chunk tile x_t -> dst[:, j, cc:cc+w]."""
                for j in js:
                    ps_p = ps.tile([128, 512], F32, tag="mm", bufs=2, name="ps_p")
                    for g in range(NG):
                        if proj_fp8:
                            nc.tensor.matmul(
                                ps_p[:, 0:w],
                                w_sb[:, g, :, j * 128:(j + 1) * 128],
                                x_t[:, g, :, 0:w],
                                perf_mode=DR,
                                start=(g == 0), stop=(g == NG - 1),
                            )
                        else:
                            nc.tensor.matmul(
                                ps_p[:, 0:w], w_sb[:, g, j * 128:(j + 1) * 128],
                                x_t[:, g, 0:w],
                                start=(g == 0), stop=(g == NG - 1),
                            )
                    nc.vector.tensor_scalar_add(
                        dst[:, j, cc:cc + w], ps_p[:, 0:w], b_sb[:, j:j + 1]
                    )

            def v_proj(kt):
                """value projection for key tile kt -> v_sb[:, kt, :]."""
                x_t = xv_sb[(kt * 128) // VW]
                c0 = (kt * 128) % VW
                ps_v = ps.tile([128, 512], F32, tag="mm", bufs=2, name="ps_v")
                for g in range(NG):
                    if proj_fp8:
                        nc.tensor.matmul(
                            ps_v[:, 0:HL],
                            x_t[:, g, :, c0:c0 + 128],
                            wv_sb[:, g, :, :],
                            perf_mode=DR,
                            start=(g == 0), stop=(g == NG - 1),
                        )
                    else:
                        nc.tensor.matmul(
                            ps_v[:, 0:HL], x_t[:, g, c0:c0 + 128],
                            wv_sb[:, g, :],
                            start=(g == 0), stop=(g == NG - 1),
                        )
                pv4 = ps_v[:, 0:HL].rearrange("p (h c) -> p h c", h=HPC)
                vb4 = bvb_sb.rearrange("p (h c) -> p h c", h=HPC)[:, :, 0:64]
                dst = v_sb[:, kt, :].rearrange("p (h c) -> p h c", h=HPC)[:, :, 0:64]
                nc.vector.tensor_add(dst, pv4, vb4)

            def qk_exp(qb, kb):
                """logits + exp, two heads per Activation instruction."""
                for jp in range(2):  # head pair (jp*2, jp*2+1)
                    lg = ps.tile([128, 1024], F32, tag="lg", bufs=2, name="lg")
                    for hh in range(2):
                        h = jp * 2 + hh
                        p0 = (h % 2) * 64
                        nc.tensor.matmul(
                            lg[:, hh * 512:(hh + 1) * 512],
                            kT[p0:p0 + 64, h // 2, kb * 128:(kb + 1) * 128],
                            qT[p0:p0 + 64, h // 2, qb * 512:(qb + 1) * 512],
                            start=True, stop=True,
                        )
                    nc.scalar.activation(
                        e_sb[:, qb % 2, kb, jp * 1024:(jp + 1) * 1024], lg[:],
                        AFT.Exp, bias=maskb_sb[:, kb:kb + 1], scale=0.125,
                    )

            cn_tiles = {}

            def pv_mm(qb, qq):
                """PV matmuls + normalize for one 128-query chunk."""
                cx = ps.tile([128, HPC * 65], F32, tag="cx", bufs=2, name="cx")
                for h in range(HPC):
                    for kb in range(kblocks):
                        nc.tensor.matmul(
                            cx[:, h * 65:(h + 1) * 65],
                            e_sb[:, qb % 2, kb,
                                 h * 512 + qq * 128:h * 512 + (qq + 1) * 128],
                            v_sb[:, kb, h * 65:(h + 1) * 65],
                            start=(kb == 0), stop=(kb == kblocks - 1),
                        )
                rcp = iop.tile([128, 4], F32, tag="rcp", bufs=3, name="rcp")
                dens = cx[:].rearrange("p (h c) -> p h c", h=HPC)[:, :, 64]
                nc.vector.reciprocal(rcp[:], dens)
                for hp in range(2):
                    cn = iop.tile([128, 128], BF16, tag="cn", bufs=8, name="cn")
                    for hh in range(2):
                        h = hp * 2 + hh
                        if qb == 3:
                            # Act is idle after the last exp
                            nc.scalar.activation(
                                cn[:, hh * 64:(hh + 1) * 64],
                                cx[:, h * 65:h * 65 + 64], AFT.Copy,
                                scale=rcp[:, h:h + 1],
                            )
                        else:
                            nc.vector.tensor_scalar_mul(
                                cn[:, hh * 64:(hh + 1) * 64],
                                cx[:, h * 65:h * 65 + 64], rcp[:, h:h + 1],
                            )
                    cn_tiles[(qq, hp)] = cn

            def pv_tr(qb, qqs):
                """transpose + copy + a2a staging for query chunks."""
                for qq in qqs:
                    ctq = iop.tile([128, 2, 128], cdt, tag="ctq", bufs=4,
                                   name="ctq")
                    for hp in range(2):
                        cn = cn_tiles.pop((qq, hp))
                        # transpose via PE into a bitcast view of an mm tile
                        tp = ps.tile([128, 512], F32, tag="mm", bufs=2, name="tp")
                        tpv = tp[:, 0:64].bitcast(BF16)
                        nc.tensor.transpose(tpv, cn[:], ident_sb[:])
                        if qb == 3:
                            # Act engine is idle after the last exp; keep the
                            # tail's mul->transpose->copy chain off the DVE
                            nc.scalar.copy(ctq[:, hp], tpv)
                        else:
                            nc.vector.tensor_copy(ctq[:, hp], tpv)
                    # stage as two 64-token chunks for the exchange
                    for s in range(2):
                        nc.sync.dma_start(
                            a2a_in[qb].ap()[2 * qq + s],
                            ctq[:, :, s * 64:(s + 1) * 64],
                        )

            def a2a_exchange(j, g0=0, g1=N_CORES):
                if collective:
                    assert (g0, g1) == (0, N_CORES)
                    nc.gpsimd.collective_compute(
                        "AllToAll",
                        mybir.AluOpType.bypass,
                        replica_groups=[list(range(N_CORES))],
                        ins=[a2a_in[j].ap().opt()],
                        outs=[a2a_out[j].ap().opt()],
                    )
                else:
                    nc.gpsimd.dma_start(
                        a2a_out[j].ap()[g0:g1], a2a_in[j].ap()[g0:g1]
                    )

            def ctxf_load(j, halves=(0, 1), ctxf=None, split_q=False):
                # free layout (g, hp, b*64+t): each (g, hp) slice is a valid
                # single-free-dim stationary operand covering both batches
                if ctxf is None:
                    ctxf = iop.tile([128, GROUP, 2, 128], cdt, tag="ctxf",
                                    bufs=2, name="ctxf")
                for b in halves:
                    for hp in range(2):
                        eng = nc.scalar if (split_q and b == 1) else nc.sync
                        eng.dma_start(
                            ctxf[:, :, hp, b * 64:(b + 1) * 64],
                            a2a_out[j].ap()[b * 4:(b + 1) * 4, :, hp].rearrange(
                                "g p t -> p g t"),
                        )
                return ctxf

            def out_proj_bb(j, ctxf, wo_sb, bob_sb):
                """out proj split by batch: batch-0 matmuls only need the
                first four exchanged chunks (ctxf cols 0:64)."""
                o_sb = iop.tile([128, D], BF16, tag="osb", bufs=2, name="o_sb")
                for half in range(2):
                    for b in range(2):
                        ps_o = ps.tile([128, 512], F32, tag="mm", bufs=2,
                                       name="ps_o")
                        n = 0
                        for g in range(GROUP):
                            for hp in range(2):
                                nc.tensor.matmul(
                                    ps_o[0:64, :],
                                    ctxf[:, g, hp, b * 64:(b + 1) * 64],
                                    wo_sb[:, g * 2 + hp,
                                          half * 512:(half + 1) * 512],
                                    start=(n == 0), stop=(n == 2 * GROUP - 1),
                                )
                                n += 1
                        nc.vector.tensor_add(
                            o_sb[b * 64:(b + 1) * 64,
                                 half * 512:(half + 1) * 512],
                            ps_o[0:64, :],
                            bob_sb[0:64, half * 512:(half + 1) * 512],
                        )
                    nc.sync.dma_start(
                        out.ap()[j, :, half * 512:(half + 1) * 512],
                        o_sb[:, half * 512:(half + 1) * 512],
                    )

            def out_proj(j, ctxf, wo_sb, bob_sb, halves=(0, 1), o_sb=None):
                # rows 0:64 = my 64-token slice of batch 0, rows 64:128 batch 1
                if o_sb is None:
                    o_sb = iop.tile([128, D], BF16, tag="osb", bufs=2,
                                    name="o_sb")
                for half in halves:
                    ps_o = ps.tile([128, 512], F32, tag="mm", bufs=2,
                                   name="ps_o")
                    n = 0
                    for g in range(GROUP):
                        for hp in range(2):
                            nc.tensor.matmul(
                                ps_o[:], ctxf[:, g, hp, :],
                                wo_sb[:, g * 2 + hp,
                                      half * 512:(half + 1) * 512],
                                start=(n == 0), stop=(n == 2 * GROUP - 1),
                            )
                            n += 1
                    nc.vector.tensor_add(
                        o_sb[:, half * 512:(half + 1) * 512], ps_o[:],
                        bob_sb[:, half * 512:(half + 1) * 512],
                    )
                    nc.sync.dma_start(
                        out.ap()[j, :, half * 512:(half + 1) * 512],
                        o_sb[:, half * 512:(half + 1) * 512],
                    )
                return o_sb

            # ---------- schedule ----------
            if out_fp8:
                wo_sb = wp.tile([128, GROUP, 2, D], F8)
            else:
                wo_sb = wp.tile([128, NDT, D], BF16)
            bob_sb = wp.tile([128, D], F32)

            for rep in range(reps):
                # ---- round 0: all projections + exp(0) stream.
                # first chunks and weights arrive in interleaved g-halves so
                # the first projection matmuls start ~4us earlier ----
                H0 = NG // 2
                if rep == 0:
                    nc.sync.dma_start(wk_sb[:, 0:H0], wk.ap()[0:H0].rearrange(wre))
                nc.sync.dma_start(xk_sb[0][:, 0:H0], xk.ap()[0, :, 0:H0])
                if rep == 0:
                    nc.sync.dma_start(wq_sb[:, 0:H0], wq.ap()[0:H0].rearrange(wre))
                nc.sync.dma_start(xq_sb[0][:, 0:H0], xq.ap()[0, :, 0:H0])
                if rep == 0:
                    nc.sync.dma_start(const_sb[:], consts.ap())
                    nc.sync.dma_start(wk_sb[:, H0:], wk.ap()[H0:].rearrange(wre))
                nc.sync.dma_start(xk_sb[0][:, H0:], xk.ap()[0, :, H0:])
                if rep == 0:
                    nc.sync.dma_start(wq_sb[:, H0:], wq.ap()[H0:].rearrange(wre))
                nc.sync.dma_start(xq_sb[0][:, H0:], xq.ap()[0, :, H0:])
                qk_proj(xk_sb[0], wk_sb, bk_sb, kT, 0, KW)
                qk_proj(xq_sb[0], wq_sb, bq_sb, qT, 0, 512)
                for kb in range(min(2, kblocks)):
                    qk_exp(0, kb)
                for c in range(1, NKC):
                    load_chunk(xk_sb, xk, c)
                if rep == 0:
                    nc.sync.dma_start(ident_sb[:], ident.ap())
                for c in range(1, NQC):
                    load_chunk(xq_sb, xq, c)
                for c in range(1, NKC):
                    qk_proj(xk_sb[c], wk_sb, bk_sb, kT, c * KW, KW)
                for kb in range(2, min(4, kblocks)):
                    qk_exp(0, kb)
                qk_proj(xq_sb[1], wq_sb, bq_sb, qT, 512, 512)
                for kb in range(4, kblocks):
                    qk_exp(0, kb)
                if rep == 0:
                    nc.sync.dma_start(wv_sb[:], wv.ap().rearrange(wre))
                for c in range(NVC):
                    load_chunk(xv_sb, xv, c)
                for kt in range(min(6, NKT)):
                    v_proj(kt)
                if rep == 0:
                    nc.sync.dma_start(
                        wo_sb[:],
                        wo.ap().rearrange(
                            "g p s m -> p g s m" if out_fp8 else "g p m -> p g m"
                        ),
                    )
                    nc.sync.dma_start(bob_sb[:], bob.ap())

                # ---- rounds 1-3: exp(r) stream hosting round r-1's
                # pv/exchange and round r-2's output projection ----
                ctxf_t = {}
                ob_t = {}
                for r in range(1, NQB):
                    qk_exp(r, 0)
                    if r == 1 and NKT > 6:
                        v_proj(6)
                    qk_exp(r, 1)
                    if r == 1 and NKT > 7:
                        v_proj(7)
                    if r >= 2:
                        cf = ctxf_t[r - 2]
                        ob_t[r - 2] = out_proj(r - 2, cf, wo_sb, bob_sb, (0,))
                    qk_exp(r, 2)
                    if r < NQB - 1:
                        qk_proj(xq_sb[r + 1], wq_sb, bq_sb, qT,
                                (r + 1) * 512, 512, js=(0,))
                    if r == 3:
                        out_proj(0, ctxf_t.pop(0), wo_sb, bob_sb, (1,),
                                 ob_t.pop(0))
                    pv_mm(r - 1, 0)
                    qk_exp(r, 3)
                    if r < NQB - 1:
                        qk_proj(xq_sb[r + 1], wq_sb, bq_sb, qT,
                                (r + 1) * 512, 512, js=(1,))
                    pv_mm(r - 1, 1)
                    pv_tr(r - 1, (0,))
                    qk_exp(r, 4)
                    pv_mm(r - 1, 2)
                    pv_tr(r - 1, (1,))
                    qk_exp(r, 5)
                    pv_mm(r - 1, 3)
                    pv_tr(r - 1, (2,))
                    qk_exp(r, 6)
                    pv_tr(r - 1, (3,))
                    if not collective:
                        a2a_exchange(r - 1, 0, 4)
                    qk_exp(r, 7)
                    if collective:
                        a2a_exchange(r - 1)
                    else:
                        a2a_exchange(r - 1, 4, 8)
                    ctxf_t[r - 1] = ctxf_load(r - 1)

                # ---- tail: last block's pv, then the two outputs ----
                for qq in range(4):
                    pv_mm(3, qq)
                    pv_tr(3, (qq,))
                    if qq == 1 and not collective:
                        a2a_exchange(3, 0, 4)
                out_proj(1, ctxf_t.pop(1), wo_sb, bob_sb, (1,), ob_t.pop(1))
                out_proj(2, ctxf_t.pop(2), wo_sb, bob_sb)
                if collective:
                    a2a_exchange(3)
                else:
                    a2a_exchange(3, 4, 8)
                out_proj(3, ctxf_load(3), wo_sb, bob_sb)

                if dump:
                    nc.sync.dma_start(d_qT.ap(), qT[:])
                    nc.sync.dma_start(d_kT.ap(), kT[:])
                    nc.sync.dma_start(d_v.ap(), v_sb[:])
                    nc.sync.dma_start(d_e.ap(), e_sb[:, 1])

    nc.compile()
    return nc


_NC_CACHE = {}


def _get_nc(key):
    if key not in _NC_CACHE:
        kblocks, proj_fp8, out_fp8 = key
        _NC_CACHE[key] = build_nc(kblocks=kblocks, proj_fp8=proj_fp8,
                                  out_fp8=out_fp8)
    return _NC_CACHE[key]


# identical on every core -> uploaded once, replicated by XLA
_REPLICATED = {"wo", "bob", "ident"}

_RUNNER_CACHE = {}


def _make_runner(nc):
    import jax
    from jax.sharding import Mesh, NamedSharding, PartitionSpec as P
    from jax.experimental.shard_map import shard_map
    import concourse.bass2jax as b2j

    b2j.install_neuronx_cc_hook()
    in_names, out_names, out_avals = [], [], []
    for alloc in nc.m.functions[0].allocations:
        if not isinstance(alloc, mybir.MemoryLocationSet):
            continue
        name = alloc.memorylocations[0].name
        if alloc.kind == "ExternalInput":
            in_names.append(name)
        elif alloc.kind == "ExternalOutput":
            out_names.append(name)
            out_avals.append(
                jax.core.ShapedArray(
                    tuple(alloc.tensor_shape), mybir.dt.np(alloc.dtype)
                )
            )
    pid_name = nc.partition_id_tensor.name if nc.partition_id_tensor else None
    all_in_names = in_names + out_names

    def _body(*args):
        return tuple(
            b2j._bass_exec_p.bind(
                *args,
                out_avals=tuple(out_avals),
                in_names=tuple(all_in_names),
                out_names=tuple(out_names),
                lowering_input_output_aliases=(),
                sim_require_finite=True,
                sim_require_nnan=True,
                nc=nc,
            )
        )

    devices = jax.devices()[:N_CORES]
    mesh = Mesh(np.asarray(devices), ("core",))

    def spec_for(name):
        return P() if name in _REPLICATED else P("core")

    in_specs = tuple(spec_for(n) for n in in_names) + (P("core"),) * len(out_names)
    out_specs = (P("core"),) * len(out_names)
    fn = jax.jit(
        shard_map(_body, mesh=mesh, in_specs=in_specs, out_specs=out_specs,
                  check_rep=False),
        keep_unused=True,
    )
    sh_core = NamedSharding(mesh, P("core"))
    sh_repl = NamedSharding(mesh, P())
    zero_outs = [
        np.zeros((N_CORES * a.shape[0],) + tuple(a.shape[1:]), a.dtype)
        for a in out_avals
    ]
    upload_cache = {}

    def _put(name, arr, sh):
        import hashlib
        key = hashlib.blake2b(arr.tobytes(), digest_size=16).digest()
        hit = upload_cache.get(name)
        if hit is not None and hit[0] == key:
            return hit[1]
        buf = jax.device_put(arr, sh)
        upload_cache[name] = (key, buf)
        return buf

    def run(in_maps):
        args = []
        for name in in_names:
            if name == pid_name:
                cat = np.arange(N_CORES, dtype=np.uint32).reshape(N_CORES, 1)
                args.append(_put(name, cat, sh_core))
            elif name in _REPLICATED:
                args.append(_put(name, np.asarray(in_maps[0][name]), sh_repl))
            else:
                cat = np.concatenate(
                    [np.asarray(m[name]) for m in in_maps], axis=0
                )
                args.append(_put(name, cat, sh_core))
        for i, z in enumerate(zero_outs):
            args.append(_put(f"__zero{i}", z, sh_core))
        outs = fn(*args)
        jax.block_until_ready(outs)
        res = []
        for c in range(N_CORES):
            d = {}
            for i, name in enumerate(out_names):
                arr = np.asarray(outs[i])
                per = arr.shape[0] // N_CORES
                d[name] = arr[c * per:(c + 1) * per]
            res.append(d)
        return res

    return run


def _get_runner(key):
    if key not in _RUNNER_CACHE:
        _RUNNER_CACHE[key] = _make_runner(_get_nc(key))
    return _RUNNER_CACHE[key]


def _dr_pack(a, ncols):
    """[D, n] fp32 -> DoubleRow layout [NDP, 128, 2, n]: row d = 256g+128s+p."""
    return np.ascontiguousarray(a.reshape(NDP, 2, 128, ncols).swapaxes(1, 2))


def _x_chunks(a, W, proj_fp8):
    """[D, n] fp32 -> chunk-major x layout [n//W, 128, ., W]."""
    n = a.shape[1]
    nch = n // W
    if proj_fp8:
        # [g, s, p, c, w] -> [c, p, g, s, w]
        r = a.reshape(NDP, 2, 128, nch, W).transpose(3, 2, 0, 1, 4)
    else:
        r = a.reshape(NDT, 128, nch, W).transpose(2, 1, 0, 3)
    return np.ascontiguousarray(r)


def prepare_in_maps(kblocks, proj_fp8, out_fp8, query, key, value, mask,
                    Wq, bq, Wk, bk, Wv, bv, Wo, bo):
    import ml_dtypes
    bf16 = ml_dtypes.bfloat16
    f8 = ml_dtypes.float8_e4m3
    xnp = f8 if proj_fp8 else bf16
    cnp = f8 if out_fp8 else bf16
    SK = kblocks * 128
    m = np.asarray(mask).reshape(B, S)

    def wpack(a):  # [D, n] fp32 weight -> device layout
        if proj_fp8:
            return _dr_pack(a, a.shape[1]).astype(xnp)
        return np.ascontiguousarray(a.reshape(NDT, 128, a.shape[1])).astype(xnp)

    KW, QW, VW = min(256, SK), 512, min(512, SK)
    xq_b, xk_b, xv_b, maskb_b = [], [], [], []
    for b in range(B):
        idx = np.flatnonzero(m[b] == 0)
        n = len(idx)
        assert n <= SK, f"unmasked count {n} exceeds capacity {SK}"
        k_b = np.zeros((SK, D), np.float32)
        v_b = np.zeros((SK, D), np.float32)
        k_b[:n] = np.asarray(key, np.float32)[b][idx]
        v_b[:n] = np.asarray(value, np.float32)[b][idx]
        xq_b.append(_x_chunks(
            np.ascontiguousarray(np.asarray(query, np.float32)[b].T), QW,
            proj_fp8).astype(xnp))
        xk_b.append(_x_chunks(
            np.ascontiguousarray(k_b.T), KW, proj_fp8).astype(xnp))
        xv_b.append(_x_chunks(
            np.ascontiguousarray(v_b.T), VW, proj_fp8).astype(xnp))
        mb = np.full((kblocks, 128), -1e9, np.float32)
        mb.reshape(-1)[:n] = 0.0
        maskb_b.append(np.ascontiguousarray(mb.T))

    Wo_f = np.asarray(Wo, np.float32)
    if out_fp8:
        Wo_c = np.ascontiguousarray(
            Wo_f.reshape(GROUP, 2, 128, D).swapaxes(1, 2)).astype(cnp)
    else:
        Wo_c = np.ascontiguousarray(Wo_f.reshape(NDT, 128, D)).astype(cnp)
    bob = np.ascontiguousarray(
        np.broadcast_to(np.asarray(bo, np.float32), (128, D)))
    ident = np.eye(128, dtype=np.float32).astype(bf16)

    in_maps = []
    for c in range(N_CORES):
        b, r = c // GROUP, c % GROUP
        sl = slice(r * HL, (r + 1) * HL)
        bv_c = np.asarray(bv, np.float32)[sl]
        bvb = np.zeros((128, HPC * 65), np.float32)
        for h in range(HPC):
            bvb[:, h * 65:h * 65 + 64] = bv_c[h * 64:(h + 1) * 64]
            bvb[:, h * 65 + 64] = 1.0
        consts = np.concatenate([
            np.ascontiguousarray(
                np.asarray(bk, np.float32)[sl].reshape(2, 128).T),
            np.ascontiguousarray(
                np.asarray(bq, np.float32)[sl].reshape(2, 128).T),
            maskb_b[b],
            bvb,
        ], axis=1)
        in_maps.append({
            "xq": xq_b[b], "xk": xk_b[b], "xv": xv_b[b],
            "wq": wpack(np.ascontiguousarray(np.asarray(Wq, np.float32)[:, sl])),
            "wk": wpack(np.ascontiguousarray(np.asarray(Wk, np.float32)[:, sl])),
            "wv": wpack(np.ascontiguousarray(np.asarray(Wv, np.float32)[:, sl])),
            "wo": Wo_c,
            "consts": np.ascontiguousarray(consts),
            "bob": bob,
            "ident": ident,
        })
    return in_maps


def _pick_kblocks(mask):
    m = np.asarray(mask).reshape(B, S)
    maxn = int((m == 0).sum(axis=1).max())
    return min(S // 128, max(1, math.ceil(maxn / 128)))


PROJ_FP8 = False
OUT_FP8 = False


def kernel(**inputs) -> np.ndarray:
    kblocks = _pick_kblocks(inputs["mask"])
    in_maps = prepare_in_maps(kblocks, PROJ_FP8, OUT_FP8, **inputs)
    key = (kblocks, PROJ_FP8, OUT_FP8)
    try:
        run = _get_runner(key)
        results = run(in_maps)
    except Exception:
        res = bass_utils.run_bass_kernel_spmd(
            _get_nc(key), in_maps, core_ids=list(range(N_CORES))
        )
        results = res.results
    out = np.zeros((B, S, D), np.float32)
    for c in range(N_CORES):
        o = np.asarray(results[c]["out"], np.float32)  # [NQB, 128, D]
        for j in range(NQB):
            for beta in range(B):
                out[beta, j * 512 + c * 64:j * 512 + (c + 1) * 64] = \
                    o[j, beta * 64:(beta + 1) * 64]
    return out


# revision 5
# speedup vs baseline: 1.0687x; 1.0139x over previous
"""Multi-head attention (B=2, S=2048, D=1024, H=16) on 8 TRN2 NeuronCores, v2.

Sharding: 2-way batch data-parallel x 4-way head tensor-parallel.
Core c handles batch c//4 with heads [4r, 4r+4) where r = c%4.

Key ideas vs v1:
- bf16/fp8 activations+weights (less HBM traffic, same-or-better PE rate).
- key compaction without the +1 safety block (exact ceil(n/128) blocks).
- transposed PV (attn^T is exactly the exp output layout): ctx comes out
  [q, d] with full 128-partition outputs -> half the PE rows of v1's PV,
  and softmax normalization becomes a cheap per-partition scalar multiply.
- denominator via an all-ones column appended to each head's v tile.
- 1024-wide exp (2 heads x 512 q per Activation instruction, reading a
  double-buffered 2-bank PSUM tile).
- fp8(e4m3) DoubleRow projections: the host ships x and W pre-interleaved
  as [128, 2-slot, .] with contraction dim d = 256*g + 128*slot + p, so a
  256-deep contraction runs at 0.5 cycles/row with no on-device shuffle.
- optional fp8 output projection: ctxT's [dim-pair, token] layout is
  already DoubleRow-compatible (slot = head-pair index).
- AllToAll in 4 per-query-block rounds, each core taking a 128-token
  slice of every block, so output projection pipelines behind attention
  on every core symmetrically.
"""
import math

import numpy as np

import concourse.mybir as mybir
import concourse.tile as tile
from concourse import bacc, bass_utils

B, S, D, H = 2, 2048, 1024, 16
DEPTH = 64
N_CORES = 8
GROUP = 4              # cores per batch (tensor parallel over heads)
HPC = H // GROUP       # 4 heads per core
HL = HPC * DEPTH       # 256 local head dims
NDT = D // 128         # 8 contraction tiles of 128
NDP = NDT // 2         # 4 double-row contraction tiles of 256
NQB = S // 512         # 4 query blocks of 512 per batch

F32 = mybir.dt.float32
BF16 = mybir.dt.bfloat16
F8 = mybir.dt.float8e4
AFT = mybir.ActivationFunctionType
DR = mybir.MatmulPerfMode.DoubleRow


def build_nc(kblocks: int = 8, proj_fp8: bool = False, out_fp8: bool = False,
             collective: bool = True, num_devices: int = N_CORES,
             dump: bool = False, reps: int = 1):
    SK = kblocks * 128
    NKT = SK // 128
    xdt = F8 if proj_fp8 else BF16
    cdt = F8 if out_fp8 else BF16
    nc = bacc.Bacc(
        "TRN2", target_bir_lowering=False, debug=False, num_devices=num_devices
    )

    # ---- I/O (per-core slices prepared by the host) ----
    # x layouts are chunk-major so one chunk = one 2-dim DMA:
    #   fp8:  [nch, 128, NDP, 2, W]; row d = 256*g + 128*slot + p
    #   bf16: [nch, 128, NDT, W]
    # chunk widths: xk 256, xq 512, xv 512.
    KW, QW, VW = min(256, SK), 512, min(512, SK)
    NKC, NQC, NVC = SK // KW, S // QW, SK // VW
    xin = ([NDP, 2] if proj_fp8 else [NDT])
    xq = nc.dram_tensor("xq", [NQC, 128] + xin + [QW], xdt, kind="ExternalInput")
    xk = nc.dram_tensor("xk", [NKC, 128] + xin + [KW], xdt, kind="ExternalInput")
    xv = nc.dram_tensor("xv", [NVC, 128] + xin + [VW], xdt, kind="ExternalInput")
    wshape = ([NDP, 128, 2] if proj_fp8 else [NDT, 128])
    wq = nc.dram_tensor("wq", wshape + [HL], xdt, kind="ExternalInput")
    wk = nc.dram_tensor("wk", wshape + [HL], xdt, kind="ExternalInput")
    wv = nc.dram_tensor("wv", wshape + [HL], xdt, kind="ExternalInput")
    # wo: fp8 [GROUP*2, 128, 2, D] with row (i, slot, p) = ctx dim
    # 256i + 128*slot + p; bf16 [NDT, 128, D] plain.
    woshape = ([GROUP, 128, 2] if out_fp8 else [NDT, 128])
    wo = nc.dram_tensor("wo", woshape + [D], cdt, kind="ExternalInput")
    # consts packed in one tensor: bk | bq | maskb | bvb (f32 columns)
    NCC = 4 + kblocks + HPC * 65
    consts = nc.dram_tensor("consts", [128, NCC], F32, kind="ExternalInput")
    bob = nc.dram_tensor("bob", [128, D], F32, kind="ExternalInput")
    ident = nc.dram_tensor("ident", [128, 128], BF16, kind="ExternalInput")
    out = nc.dram_tensor("out", [NQB, 128, D], BF16, kind="ExternalOutput")

    if dump:
        d_qT = nc.dram_tensor("d_qT", [128, 2, S], BF16, kind="ExternalOutput")
        d_kT = nc.dram_tensor("d_kT", [128, 2, SK], BF16, kind="ExternalOutput")
        d_v = nc.dram_tensor("d_v", [128, NKT, HPC * 65], BF16,
                             kind="ExternalOutput")
        d_e = nc.dram_tensor("d_e", [128, kblocks, HPC * 512], BF16,
                             kind="ExternalOutput")

    # global 8-way AllToAll: chunk g = my 256 dims for tokens
    # [qb*512 + g*64, +64) of my batch; core c ends up with all 1024 dims of
    # BOTH batches' 64-token slice c.
    a2a_in = [nc.dram_tensor(f"a2a_in{j}", [N_CORES, 128, 2, 64], cdt)
              for j in range(NQB)]
    a2a_out = [nc.dram_tensor(f"a2a_out{j}", [N_CORES, 128, 2, 64], cdt)
               for j in range(NQB)]

    with tile.TileContext(nc) as tc:
        with (
            tc.tile_pool(name="w", bufs=1) as wp,
            tc.tile_pool(name="big", bufs=1) as bigp,
            tc.tile_pool(name="io", bufs=4) as iop,
            tc.tile_pool(name="ps", bufs=1, space="PSUM") as ps,
        ):
            # ---- constants ----
            wsl = [NDP, 2] if proj_fp8 else [NDT]
            wq_sb = wp.tile([128] + wsl + [HL], xdt)
            wk_sb = wp.tile([128] + wsl + [HL], xdt)
            wv_sb = wp.tile([128] + wsl + [HL], xdt)
            wre = "g p s m -> p g s m" if proj_fp8 else "g p m -> p g m"
            const_sb = wp.tile([128, NCC], F32)
            bk_sb = const_sb[:, 0:2]
            bq_sb = const_sb[:, 2:4]
            maskb_sb = const_sb[:, 4:4 + kblocks]
            bvb_sb = const_sb[:, 4 + kblocks:4 + kblocks + HPC * 65]
            ident_sb = wp.tile([128, 128], BF16)

            # ---- persistent activations (one tile per input chunk) ----
            xsl = [NDP, 2] if proj_fp8 else [NDT]
            xq_sb = [bigp.tile([128] + xsl + [QW], xdt, name=f"xq{i}")
                     for i in range(NQC)]
            xk_sb = [bigp.tile([128] + xsl + [KW], xdt, name=f"xk{i}")
                     for i in range(NKC)]
            xv_sb = [bigp.tile([128] + xsl + [VW], xdt, name=f"xv{i}")
                     for i in range(NVC)]
            qT = bigp.tile([128, 2, S], BF16)    # local q dim j*128+p
            kT = bigp.tile([128, 2, SK], BF16)
            v_sb = bigp.tile([128, NKT, HPC * 65], BF16)
            e_sb = bigp.tile([128, 2, kblocks, HPC * 512], BF16)

            # ones columns (denominator trick): v_sb[:, :, h*65+64] = 1
            nc.vector.memset(
                v_sb[:].rearrange("p t (h c) -> p t h c", h=HPC)[:, :, :, 64], 1.0
            )

            # warm the Exp activation table off the critical path
            warm = iop.tile([1, 1], F32, tag="warm", bufs=1, name="warm")
            nc.scalar.activation(warm[:], warm[:], AFT.Exp)

            # ---------- emission helpers ----------
            def load_chunk(x_sb, x_dram, c):
                nc.sync.dma_start(x_sb[c][:], x_dram.ap()[c])

            NG = NDP if proj_fp8 else NDT

            def qk_proj(x_t, w_sb, b_sb, dst, cc, w, js=(0, 1)):
                """project chunk tile x_t -> dst[:, j, cc:cc+w]."""
                for j in js:
                    ps_p = ps.tile([128, 512], F32, tag="mm", bufs=2, name="ps_p")
                    for g in range(NG):
                        if proj_fp8:
                            nc.tensor.matmul(
                                ps_p[:, 0:w],
                                w_sb[:, g, :, j * 128:(j + 1) * 128],
                                x_t[:, g, :, 0:w],
                                perf_mode=DR,
                                start=(g == 0), stop=(g == NG - 1),
                            )
                        else:
                            nc.tensor.matmul(
                                ps_p[:, 0:w], w_sb[:, g, j * 128:(j + 1) * 128],
                                x_t[:, g, 0:w],
                                start=(g == 0), stop=(g == NG - 1),
                            )
                    nc.vector.tensor_scalar_add(
                        dst[:, j, cc:cc + w], ps_p[:, 0:w], b_sb[:, j:j + 1]
                    )

            def v_proj(kt):
                """value projection for key tile kt -> v_sb[:, kt, :]."""
                x_t = xv_sb[(kt * 128) // VW]
                c0 = (kt * 128) % VW
                ps_v = ps.tile([128, 512], F32, tag="mm", bufs=2, name="ps_v")
                for g in range(NG):
                    if proj_fp8:
                        nc.tensor.matmul(
                            ps_v[:, 0:HL],
                            x_t[:, g, :, c0:c0 + 128],
                            wv_sb[:, g, :, :],
                            perf_mode=DR,
                            start=(g == 0), stop=(g == NG - 1),
                        )
                    else:
                        nc.tensor.matmul(
                            ps_v[:, 0:HL], x_t[:, g, c0:c0 + 128],
                            wv_sb[:, g, :],
                            start=(g == 0), stop=(g == NG - 1),
                        )
                pv4 = ps_v[:, 0:HL].rearrange("p (h c) -> p h c", h=HPC)
                vb4 = bvb_sb.rearrange("p (h c) -> p h c", h=HPC)[:, :, 0:64]
                dst = v_sb[:, kt, :].rearrange("p (h c) -> p h c", h=HPC)[:, :, 0:64]
                nc.vector.tensor_add(dst, pv4, vb4)

            def qk_exp(qb, kb):
                """logits + exp, two heads per Activation instruction."""
                for jp in range(2):  # head pair (jp*2, jp*2+1)
                    lg = ps.tile([128, 1024], F32, tag="lg", bufs=2, name="lg")
                    for hh in range(2):
                        h = jp * 2 + hh
                        p0 = (h % 2) * 64
                        nc.tensor.matmul(
                            lg[:, hh * 512:(hh + 1) * 512],
                            kT[p0:p0 + 64, h // 2, kb * 128:(kb + 1) * 128],
                            qT[p0:p0 + 64, h // 2, qb * 512:(qb + 1) * 512],
                            start=True, stop=True,
                        )
                    nc.scalar.activation(
                        e_sb[:, qb % 2, kb, jp * 1024:(jp + 1) * 1024], lg[:],
                        AFT.Exp, bias=maskb_sb[:, kb:kb + 1], scale=0.125,
                    )

            cn_tiles = {}

            def pv_mm(qb, qq):
                """PV matmuls + normalize for one 128-query chunk."""
                cx = ps.tile([128, HPC * 65], F32, tag="cx", bufs=2, name="cx")
                for h in range(HPC):
                    for kb in range(kblocks):
                        nc.tensor.matmul(
                            cx[:, h * 65:(h + 1) * 65],
                            e_sb[:, qb % 2, kb,
                                 h * 512 + qq * 128:h * 512 + (qq + 1) * 128],
                            v_sb[:, kb, h * 65:(h + 1) * 65],
                            start=(kb == 0), stop=(kb == kblocks - 1),
                        )
                rcp = iop.tile([128, 4], F32, tag="rcp", bufs=3, name="rcp")
                dens = cx[:].rearrange("p (h c) -> p h c", h=HPC)[:, :, 64]
                nc.vector.reciprocal(rcp[:], dens)
                for hp in range(2):
                    cn = iop.tile([128, 128], BF16, tag="cn", bufs=8, name="cn")
                    for hh in range(2):
                        h = hp * 2 + hh
                        nc.vector.tensor_scalar_mul(
                            cn[:, hh * 64:(hh + 1) * 64],
                            cx[:, h * 65:h * 65 + 64], rcp[:, h:h + 1],
                        )
                    cn_tiles[(qq, hp)] = cn

            def pv_tr(qb, qqs):
                """transpose + copy + a2a staging for query chunks."""
                for qq in qqs:
                    ctq = iop.tile([128, 2, 128], cdt, tag="ctq", bufs=4,
                                   name="ctq")
                    for hp in range(2):
                        cn = cn_tiles.pop((qq, hp))
                        # transpose via PE into a bitcast view of an mm tile
                        tp = ps.tile([128, 512], F32, tag="mm", bufs=2, name="tp")
                        tpv = tp[:, 0:64].bitcast(BF16)
                        nc.tensor.transpose(tpv, cn[:], ident_sb[:])
                        if qb == 3:
                            # Act engine is idle after the last exp; keep the
                            # tail's transpose->copy chain off the DVE
                            nc.scalar.copy(ctq[:, hp], tpv)
                        else:
                            nc.vector.tensor_copy(ctq[:, hp], tpv)
                    # stage as two 64-token chunks for the exchange
                    for s in range(2):
                        nc.sync.dma_start(
                            a2a_in[qb].ap()[2 * qq + s],
                            ctq[:, :, s * 64:(s + 1) * 64],
                        )

            def a2a_exchange(j, g0=0, g1=N_CORES):
                if collective:
                    assert (g0, g1) == (0, N_CORES)
                    nc.gpsimd.collective_compute(
                        "AllToAll",
                        mybir.AluOpType.bypass,
                        replica_groups=[list(range(N_CORES))],
                        ins=[a2a_in[j].ap().opt()],
                        outs=[a2a_out[j].ap().opt()],
                    )
                else:
                    nc.gpsimd.dma_start(
                        a2a_out[j].ap()[g0:g1], a2a_in[j].ap()[g0:g1]
                    )

            def ctxf_load(j, halves=(0, 1), ctxf=None, split_q=False):
                # free layout (g, hp, b*64+t): each (g, hp) slice is a valid
                # single-free-dim stationary operand covering both batches
                if ctxf is None:
                    ctxf = iop.tile([128, GROUP, 2, 128], cdt, tag="ctxf",
                                    bufs=2, name="ctxf")
                for b in halves:
                    for hp in range(2):
                        eng = nc.scalar if (split_q and b == 1) else nc.sync
                        eng.dma_start(
                            ctxf[:, :, hp, b * 64:(b + 1) * 64],
                            a2a_out[j].ap()[b * 4:(b + 1) * 4, :, hp].rearrange(
                                "g p t -> p g t"),
                        )
                return ctxf

            def out_proj_bb(j, ctxf, wo_sb, bob_sb):
                """out proj split by batch: batch-0 matmuls only need the
                first four exchanged chunks (ctxf cols 0:64)."""
                o_sb = iop.tile([128, D], BF16, tag="osb", bufs=2, name="o_sb")
                for half in range(2):
                    for b in range(2):
                        ps_o = ps.tile([128, 512], F32, tag="mm", bufs=2,
                                       name="ps_o")
                        n = 0
                        for g in range(GROUP):
                            for hp in range(2):
                                nc.tensor.matmul(
                                    ps_o[0:64, :],
                                    ctxf[:, g, hp, b * 64:(b + 1) * 64],
                                    wo_sb[:, g * 2 + hp,
                                          half * 512:(half + 1) * 512],
                                    start=(n == 0), stop=(n == 2 * GROUP - 1),
                                )
                                n += 1
                        nc.vector.tensor_add(
                            o_sb[b * 64:(b + 1) * 64,
                                 half * 512:(half + 1) * 512],
                            ps_o[0:64, :],
                            bob_sb[0:64, half * 512:(half + 1) * 512],
                        )
                    nc.sync.dma_start(
                        out.ap()[j, :, half * 512:(half + 1) * 512],
                        o_sb[:, half * 512:(half + 1) * 512],
                    )

            def out_proj(j, ctxf, wo_sb, bob_sb, halves=(0, 1), o_sb=None):
                # rows 0:64 = my 64-token slice of batch 0, rows 64:128 batch 1
                if o_sb is None:
                    o_sb = iop.tile([128, D], BF16, tag="osb", bufs=2,
                                    name="o_sb")
                for half in halves:
                    ps_o = ps.tile([128, 512], F32, tag="mm", bufs=2,
                                   name="ps_o")
                    n = 0
                    for g in range(GROUP):
                        for hp in range(2):
                            nc.tensor.matmul(
                                ps_o[:], ctxf[:, g, hp, :],
                                wo_sb[:, g * 2 + hp,
                                      half * 512:(half + 1) * 512],
                                start=(n == 0), stop=(n == 2 * GROUP - 1),
                            )
                            n += 1
                    nc.vector.tensor_add(
                        o_sb[:, half * 512:(half + 1) * 512], ps_o[:],
                        bob_sb[:, half * 512:(half + 1) * 512],
                    )
                    nc.sync.dma_start(
                        out.ap()[j, :, half * 512:(half + 1) * 512],
                        o_sb[:, half * 512:(half + 1) * 512],
                    )
                return o_sb

            # ---------- schedule ----------
            if out_fp8:
                wo_sb = wp.tile([128, GROUP, 2, D], F8)
            else:
                wo_sb = wp.tile([128, NDT, D], BF16)
            bob_sb = wp.tile([128, D], F32)

            for rep in range(reps):
                # ---- round 0: all projections + exp(0) stream.
                # first chunks and weights arrive in interleaved g-halves so
                # the first projection matmuls start ~4us earlier ----
                H0 = NG // 2
                if rep == 0:
                    nc.sync.dma_start(wk_sb[:, 0:H0], wk.ap()[0:H0].rearrange(wre))
                nc.sync.dma_start(xk_sb[0][:, 0:H0], xk.ap()[0, :, 0:H0])
                if rep == 0:
                    nc.sync.dma_start(wq_sb[:, 0:H0], wq.ap()[0:H0].rearrange(wre))
                nc.sync.dma_start(xq_sb[0][:, 0:H0], xq.ap()[0, :, 0:H0])
                if rep == 0:
                    nc.sync.dma_start(const_sb[:], consts.ap())
                    nc.sync.dma_start(wk_sb[:, H0:], wk.ap()[H0:].rearrange(wre))
                nc.sync.dma_start(xk_sb[0][:, H0:], xk.ap()[0, :, H0:])
                if rep == 0:
                    nc.sync.dma_start(wq_sb[:, H0:], wq.ap()[H0:].rearrange(wre))
                nc.sync.dma_start(xq_sb[0][:, H0:], xq.ap()[0, :, H0:])
                qk_proj(xk_sb[0], wk_sb, bk_sb, kT, 0, KW)
                qk_proj(xq_sb[0], wq_sb, bq_sb, qT, 0, 512)
                for kb in range(min(2, kblocks)):
                    qk_exp(0, kb)
                for c in range(1, NKC):
                    load_chunk(xk_sb, xk, c)
                if rep == 0:
                    nc.sync.dma_start(ident_sb[:], ident.ap())
                for c in range(1, NQC):
                    load_chunk(xq_sb, xq, c)
                for c in range(1, NKC):
                    qk_proj(xk_sb[c], wk_sb, bk_sb, kT, c * KW, KW)
                for kb in range(2, min(4, kblocks)):
                    qk_exp(0, kb)
                qk_proj(xq_sb[1], wq_sb, bq_sb, qT, 512, 512)
                for kb in range(4, kblocks):
                    qk_exp(0, kb)
                if rep == 0:
                    nc.sync.dma_start(wv_sb[:], wv.ap().rearrange(wre))
                for c in range(NVC):
                    load_chunk(xv_sb, xv, c)
                for kt in range(min(6, NKT)):
                    v_proj(kt)
                if rep == 0:
                    nc.sync.dma_start(
                        wo_sb[:],
                        wo.ap().rearrange(
                            "g p s m -> p g s m" if out_fp8 else "g p m -> p g m"
                        ),
                    )
                    nc.sync.dma_start(bob_sb[:], bob.ap())

                # ---- rounds 1-3: exp(r) stream hosting round r-1's
                # pv/exchange and round r-2's output projection ----
                ctxf_t = {}
                ob_t = {}
                for r in range(1, NQB):
                    qk_exp(r, 0)
                    if r == 1 and NKT > 6:
                        v_proj(6)
                    qk_exp(r, 1)
                    if r == 1 and NKT > 7:
                        v_proj(7)
                    if r >= 2:
                        cf = ctxf_t[r - 2]
                        ob_t[r - 2] = out_proj(r - 2, cf, wo_sb, bob_sb, (0,))
                    qk_exp(r, 2)
                    if r < NQB - 1:
                        qk_proj(xq_sb[r + 1], wq_sb, bq_sb, qT,
                                (r + 1) * 512, 512, js=(0,))
                    if r == 3:
                        out_proj(0, ctxf_t.pop(0), wo_sb, bob_sb, (1,),
                                 ob_t.pop(0))
                    pv_mm(r - 1, 0)
                    qk_exp(r, 3)
                    if r < NQB - 1:
                        qk_proj(xq_sb[r + 1], wq_sb, bq_sb, qT,
                                (r + 1) * 512, 512, js=(1,))
                    pv_mm(r - 1, 1)
                    pv_tr(r - 1, (0,))
                    qk_exp(r, 4)
                    pv_mm(r - 1, 2)
                    pv_tr(r - 1, (1,))
                    qk_exp(r, 5)
                    pv_mm(r - 1, 3)
                    pv_tr(r - 1, (2,))
                    qk_exp(r, 6)
                    pv_tr(r - 1, (3,))
                    if not collective:
                        a2a_exchange(r - 1, 0, 4)
                    qk_exp(r, 7)
                    if collective:
                        a2a_exchange(r - 1)
                    else:
                        a2a_exchange(r - 1, 4, 8)
                    ctxf_t[r - 1] = ctxf_load(r - 1)

                # ---- tail: last block's pv, then the two outputs ----
                for qq in range(4):
                    pv_mm(3, qq)
                    pv_tr(3, (qq,))
                    if qq == 1 and not collective:
                        a2a_exchange(3, 0, 4)
                out_proj(1, ctxf_t.pop(1), wo_sb, bob_sb, (1,), ob_t.pop(1))
                out_proj(2, ctxf_t.pop(2), wo_sb, bob_sb)
                if collective:
                    a2a_exchange(3)
                else:
                    a2a_exchange(3, 4, 8)
                out_proj(3, ctxf_load(3), wo_sb, bob_sb)

                if dump:
                    nc.sync.dma_start(d_qT.ap(), qT[:])
                    nc.sync.dma_start(d_kT.ap(), kT[:])
                    nc.sync.dma_start(d_v.ap(), v_sb[:])
                    nc.sync.dma_start(d_e.ap(), e_sb[:, 1])

    nc.compile()
    return nc


_NC_CACHE = {}


def _get_nc(key):
    if key not in _NC_CACHE:
        kblocks, proj_fp8, out_fp8 = key
        _NC_CACHE[key] = build_nc(kblocks=kblocks, proj_fp8=proj_fp8,
                                  out_fp8=out_fp8)
    return _NC_CACHE[key]


# identical on every core -> uploaded once, replicated by XLA
_REPLICATED = {"wo", "bob", "ident"}

_RUNNER_CACHE = {}


def _make_runner(nc):
    import jax
    from jax.sharding import Mesh, NamedSharding, PartitionSpec as P
    from jax.experimental.shard_map import shard_map
    import concourse.bass2jax as b2j

    b2j.install_neuronx_cc_hook()
    in_names, out_names, out_avals = [], [], []
    for alloc in nc.m.functions[0].allocations:
        if not isinstance(alloc, mybir.MemoryLocationSet):
            continue
        name = alloc.memorylocations[0].name
        if alloc.kind == "ExternalInput":
            in_names.append(name)
        elif alloc.kind == "ExternalOutput":
            out_names.append(name)
            out_avals.append(
                jax.core.ShapedArray(
                    tuple(alloc.tensor_shape), mybir.dt.np(alloc.dtype)
                )
            )
    pid_name = nc.partition_id_tensor.name if nc.partition_id_tensor else None
    all_in_names = in_names + out_names

    def _body(*args):
        return tuple(
            b2j._bass_exec_p.bind(
                *args,
                out_avals=tuple(out_avals),
                in_names=tuple(all_in_names),
                out_names=tuple(out_names),
                lowering_input_output_aliases=(),
                sim_require_finite=True,
                sim_require_nnan=True,
                nc=nc,
            )
        )

    devices = jax.devices()[:N_CORES]
    mesh = Mesh(np.asarray(devices), ("core",))

    def spec_for(name):
        return P() if name in _REPLICATED else P("core")

    in_specs = tuple(spec_for(n) for n in in_names) + (P("core"),) * len(out_names)
    out_specs = (P("core"),) * len(out_names)
    fn = jax.jit(
        shard_map(_body, mesh=mesh, in_specs=in_specs, out_specs=out_specs,
                  check_rep=False),
        keep_unused=True,
    )
    sh_core = NamedSharding(mesh, P("core"))
    sh_repl = NamedSharding(mesh, P())
    zero_outs = [
        np.zeros((N_CORES * a.shape[0],) + tuple(a.shape[1:]), a.dtype)
        for a in out_avals
    ]
    upload_cache = {}

    def _put(name, arr, sh):
        import hashlib
        key = hashlib.blake2b(arr.tobytes(), digest_size=16).digest()
        hit = upload_cache.get(name)
        if hit is not None and hit[0] == key:
            return hit[1]
        buf = jax.device_put(arr, sh)
        upload_cache[name] = (key, buf)
        return buf

    def run(in_maps):
        args = []
        for name in in_names:
            if name == pid_name:
                cat = np.arange(N_CORES, dtype=np.uint32).reshape(N_CORES, 1)
                args.append(_put(name, cat, sh_core))
            elif name in _REPLICATED:
                args.append(_put(name, np.asarray(in_maps[0][name]), sh_repl))
            else:
                cat = np.concatenate(
                    [np.asarray(m[name]) for m in in_maps], axis=0
                )
                args.append(_put(name, cat, sh_core))
        for i, z in enumerate(zero_outs):
            args.append(_put(f"__zero{i}", z, sh_core))
        outs = fn(*args)
        jax.block_until_ready(outs)
        res = []
        for c in range(N_CORES):
            d = {}
            for i, name in enumerate(out_names):
                arr = np.asarray(outs[i])
                per = arr.shape[0] // N_CORES
                d[name] = arr[c * per:(c + 1) * per]
            res.append(d)
        return res

    return run


def _get_runner(key):
    if key not in _RUNNER_CACHE:
        _RUNNER_CACHE[key] = _make_runner(_get_nc(key))
    return _RUNNER_CACHE[key]


def _dr_pack(a, ncols):
    """[D, n] fp32 -> DoubleRow layout [NDP, 128, 2, n]: row d = 256g+128s+p."""
    return np.ascontiguousarray(a.reshape(NDP, 2, 128, ncols).swapaxes(1, 2))


def _x_chunks(a, W, proj_fp8):
    """[D, n] fp32 -> chunk-major x layout [n//W, 128, ., W]."""
    n = a.shape[1]
    nch = n // W
    if proj_fp8:
        # [g, s, p, c, w] -> [c, p, g, s, w]
        r = a.reshape(NDP, 2, 128, nch, W).transpose(3, 2, 0, 1, 4)
    else:
        r = a.reshape(NDT, 128, nch, W).transpose(2, 1, 0, 3)
    return np.ascontiguousarray(r)


def prepare_in_maps(kblocks, proj_fp8, out_fp8, query, key, value, mask,
                    Wq, bq, Wk, bk, Wv, bv, Wo, bo):
    import ml_dtypes
    bf16 = ml_dtypes.bfloat16
    f8 = ml_dtypes.float8_e4m3
    xnp = f8 if proj_fp8 else bf16
    cnp = f8 if out_fp8 else bf16
    SK = kblocks * 128
    m = np.asarray(mask).reshape(B, S)

    def wpack(a):  # [D, n] fp32 weight -> device layout
        if proj_fp8:
            return _dr_pack(a, a.shape[1]).astype(xnp)
        return np.ascontiguousarray(a.reshape(NDT, 128, a.shape[1])).astype(xnp)

    KW, QW, VW = min(256, SK), 512, min(512, SK)
    xq_b, xk_b, xv_b, maskb_b = [], [], [], []
    for b in range(B):
        idx = np.flatnonzero(m[b] == 0)
        n = len(idx)
        assert n <= SK, f"unmasked count {n} exceeds capacity {SK}"
        k_b = np.zeros((SK, D), np.float32)
        v_b = np.zeros((SK, D), np.float32)
        k_b[:n] = np.asarray(key, np.float32)[b][idx]
        v_b[:n] = np.asarray(value, np.float32)[b][idx]
        xq_b.append(_x_chunks(
            np.ascontiguousarray(np.asarray(query, np.float32)[b].T), QW,
            proj_fp8).astype(xnp))
        xk_b.append(_x_chunks(
            np.ascontiguousarray(k_b.T), KW, proj_fp8).astype(xnp))
        xv_b.append(_x_chunks(
            np.ascontiguousarray(v_b.T), VW, proj_fp8).astype(xnp))
        mb = np.full((kblocks, 128), -1e9, np.float32)
        mb.reshape(-1)[:n] = 0.0
        maskb_b.append(np.ascontiguousarray(mb.T))

    Wo_f = np.asarray(Wo, np.float32)
    if out_fp8:
        Wo_c = np.ascontiguousarray(
            Wo_f.reshape(GROUP, 2, 128, D).swapaxes(1, 2)).astype(cnp)
    else:
        Wo_c = np.ascontiguousarray(Wo_f.reshape(NDT, 128, D)).astype(cnp)
    bob = np.ascontiguousarray(
        np.broadcast_to(np.asarray(bo, np.float32), (128, D)))
    ident = np.eye(128, dtype=np.float32).astype(bf16)

    in_maps = []
    for c in range(N_CORES):
        b, r = c // GROUP, c % GROUP
        sl = slice(r * HL, (r + 1) * HL)
        bv_c = np.asarray(bv, np.float32)[sl]
        bvb = np.zeros((128, HPC * 65), np.float32)
        for h in range(HPC):
            bvb[:, h * 65:h * 65 + 64] = bv_c[h * 64:(h + 1) * 64]
            bvb[:, h * 65 + 64] = 1.0
        consts = np.concatenate([
            np.ascontiguousarray(
                np.asarray(bk, np.float32)[sl].reshape(2, 128).T),
            np.ascontiguousarray(
                np.asarray(bq, np.float32)[sl].reshape(2, 128).T),
            maskb_b[b],
            bvb,
        ], axis=1)
        in_maps.append({
            "xq": xq_b[b], "xk": xk_b[b], "xv": xv_b[b],
            "wq": wpack(np.ascontiguousarray(np.asarray(Wq, np.float32)[:, sl])),
            "wk": wpack(np.ascontiguousarray(np.asarray(Wk, np.float32)[:, sl])),
            "wv": wpack(np.ascontiguousarray(np.asarray(Wv, np.float32)[:, sl])),
            "wo": Wo_c,
            "consts": np.ascontiguousarray(consts),
            "bob": bob,
            "ident": ident,
        })
    return in_maps


def _pick_kblocks(mask):
    m = np.asarray(mask).reshape(B, S)
    maxn = int((m == 0).sum(axis=1).max())
    return min(S // 128, max(1, math.ceil(maxn / 128)))


PROJ_FP8 = False
OUT_FP8 = False


def kernel(**inputs) -> np.ndarray:
    kblocks = _pick_kblocks(inputs["mask"])
    in_maps = prepare_in_maps(kblocks, PROJ_FP8, OUT_FP8, **inputs)
    key = (kblocks, PROJ_FP8, OUT_FP8)
    try:
        run = _get_runner(key)
        results = run(in_maps)
    except Exception:
        res = bass_utils.run_bass_kernel_spmd(
            _get_nc(key), in_maps, core_ids=list(range(N_CORES))
        )
        results = res.results
    out = np.zeros((B, S, D), np.float32)
    for c in range(N_CORES):
        o = np.asarray(results[c]["out"], np.float32)  # [NQB, 128, D]
        for j in range(NQB):
            for beta in range(B):
                out[beta, j * 512 + c * 64:j * 512 + (c + 1) * 64] = \
                    o[j, beta * 64:(beta + 1) * 64]
    return out


# revision 6
# speedup vs baseline: 1.0690x; 1.0003x over previous
"""Multi-head attention (B=2, S=2048, D=1024, H=16) on 8 TRN2 NeuronCores, v2.

Sharding: 2-way batch data-parallel x 4-way head tensor-parallel.
Core c handles batch c//4 with heads [4r, 4r+4) where r = c%4.

Key ideas vs v1:
- bf16/fp8 activations+weights (less HBM traffic, same-or-better PE rate).
- key compaction without the +1 safety block (exact ceil(n/128) blocks).
- transposed PV (attn^T is exactly the exp output layout): ctx comes out
  [q, d] with full 128-partition outputs -> half the PE rows of v1's PV,
  and softmax normalization becomes a cheap per-partition scalar multiply.
- denominator via an all-ones column appended to each head's v tile.
- 1024-wide exp (2 heads x 512 q per Activation instruction, reading a
  double-buffered 2-bank PSUM tile).
- fp8(e4m3) DoubleRow projections: the host ships x and W pre-interleaved
  as [128, 2-slot, .] with contraction dim d = 256*g + 128*slot + p, so a
  256-deep contraction runs at 0.5 cycles/row with no on-device shuffle.
- optional fp8 output projection: ctxT's [dim-pair, token] layout is
  already DoubleRow-compatible (slot = head-pair index).
- AllToAll in 4 per-query-block rounds, each core taking a 128-token
  slice of every block, so output projection pipelines behind attention
  on every core symmetrically.
"""
import math

import numpy as np

import concourse.mybir as mybir
import concourse.tile as tile
from concourse import bacc, bass_utils

B, S, D, H = 2, 2048, 1024, 16
DEPTH = 64
N_CORES = 8
GROUP = 4              # cores per batch (tensor parallel over heads)
HPC = H // GROUP       # 4 heads per core
HL = HPC * DEPTH       # 256 local head dims
NDT = D // 128         # 8 contraction tiles of 128
NDP = NDT // 2         # 4 double-row contraction tiles of 256
NQB = S // 512         # 4 query blocks of 512 per batch

F32 = mybir.dt.float32
BF16 = mybir.dt.bfloat16
F8 = mybir.dt.float8e4
AFT = mybir.ActivationFunctionType
DR = mybir.MatmulPerfMode.DoubleRow


def build_nc(kblocks: int = 8, proj_fp8: bool = False, out_fp8: bool = False,
             collective: bool = True, num_devices: int = N_CORES,
             dump: bool = False, reps: int = 1):
    SK = kblocks * 128
    NKT = SK // 128
    xdt = F8 if proj_fp8 else BF16
    cdt = F8 if out_fp8 else BF16
    nc = bacc.Bacc(
        "TRN2", target_bir_lowering=False, debug=False, num_devices=num_devices
    )

    # ---- I/O (per-core slices prepared by the host) ----
    # x layouts are chunk-major so one chunk = one 2-dim DMA:
    #   fp8:  [nch, 128, NDP, 2, W]; row d = 256*g + 128*slot + p
    #   bf16: [nch, 128, NDT, W]
    # chunk widths: xk 256, xq 512, xv 512.
    KW, QW, VW = min(256, SK), 512, min(512, SK)
    NKC, NQC, NVC = SK // KW, S // QW, SK // VW
    xin = ([NDP, 2] if proj_fp8 else [NDT])
    xq = nc.dram_tensor("xq", [NQC, 128] + xin + [QW], xdt, kind="ExternalInput")
    xk = nc.dram_tensor("xk", [NKC, 128] + xin + [KW], xdt, kind="ExternalInput")
    xv = nc.dram_tensor("xv", [NVC, 128] + xin + [VW], xdt, kind="ExternalInput")
    wshape = ([NDP, 128, 2] if proj_fp8 else [NDT, 128])
    wq = nc.dram_tensor("wq", wshape + [HL], xdt, kind="ExternalInput")
    wk = nc.dram_tensor("wk", wshape + [HL], xdt, kind="ExternalInput")
    wv = nc.dram_tensor("wv", wshape + [HL], xdt, kind="ExternalInput")
    # wo: fp8 [GROUP*2, 128, 2, D] with row (i, slot, p) = ctx dim
    # 256i + 128*slot + p; bf16 [NDT, 128, D] plain.
    woshape = ([GROUP, 128, 2] if out_fp8 else [NDT, 128])
    wo = nc.dram_tensor("wo", woshape + [D], cdt, kind="ExternalInput")
    # consts packed in one tensor: bk | bq | maskb | bvb (f32 columns)
    NCC = 4 + kblocks + HPC * 65
    consts = nc.dram_tensor("consts", [128, NCC], F32, kind="ExternalInput")
    bob = nc.dram_tensor("bob", [128, D], F32, kind="ExternalInput")
    ident = nc.dram_tensor("ident", [128, 128], BF16, kind="ExternalInput")
    out = nc.dram_tensor("out", [NQB, 128, D], BF16, kind="ExternalOutput")

    if dump:
        d_qT = nc.dram_tensor("d_qT", [128, 2, S], BF16, kind="ExternalOutput")
        d_kT = nc.dram_tensor("d_kT", [128, 2, SK], BF16, kind="ExternalOutput")
        d_v = nc.dram_tensor("d_v", [128, NKT, HPC * 65], BF16,
                             kind="ExternalOutput")
        d_e = nc.dram_tensor("d_e", [128, kblocks, HPC * 512], BF16,
                             kind="ExternalOutput")

    # global 8-way AllToAll: chunk g = my 256 dims for tokens
    # [qb*512 + g*64, +64) of my batch; core c ends up with all 1024 dims of
    # BOTH batches' 64-token slice c.
    a2a_in = [nc.dram_tensor(f"a2a_in{j}", [N_CORES, 128, 2, 64], cdt)
              for j in range(NQB)]
    a2a_out = [nc.dram_tensor(f"a2a_out{j}", [N_CORES, 128, 2, 64], cdt)
               for j in range(NQB)]

    with tile.TileContext(nc) as tc:
        with (
            tc.tile_pool(name="w", bufs=1) as wp,
            tc.tile_pool(name="big", bufs=1) as bigp,
            tc.tile_pool(name="io", bufs=4) as iop,
            tc.tile_pool(name="ps", bufs=1, space="PSUM") as ps,
        ):
            # ---- constants ----
            wsl = [NDP, 2] if proj_fp8 else [NDT]
            wq_sb = wp.tile([128] + wsl + [HL], xdt)
            wk_sb = wp.tile([128] + wsl + [HL], xdt)
            wv_sb = wp.tile([128] + wsl + [HL], xdt)
            wre = "g p s m -> p g s m" if proj_fp8 else "g p m -> p g m"
            const_sb = wp.tile([128, NCC], F32)
            bk_sb = const_sb[:, 0:2]
            bq_sb = const_sb[:, 2:4]
            maskb_sb = const_sb[:, 4:4 + kblocks]
            bvb_sb = const_sb[:, 4 + kblocks:4 + kblocks + HPC * 65]
            ident_sb = wp.tile([128, 128], BF16)

            # ---- persistent activations (one tile per input chunk) ----
            xsl = [NDP, 2] if proj_fp8 else [NDT]
            xq_sb = [bigp.tile([128] + xsl + [QW], xdt, name=f"xq{i}")
                     for i in range(NQC)]
            xk_sb = [bigp.tile([128] + xsl + [KW], xdt, name=f"xk{i}")
                     for i in range(NKC)]
            xv_sb = [bigp.tile([128] + xsl + [VW], xdt, name=f"xv{i}")
                     for i in range(NVC)]
            qT = bigp.tile([128, 2, S], BF16)    # local q dim j*128+p
            kT = bigp.tile([128, 2, SK], BF16)
            v_sb = bigp.tile([128, NKT, HPC * 65], BF16)
            e_sb = bigp.tile([128, 2, kblocks, HPC * 512], BF16)

            # ones columns (denominator trick): v_sb[:, :, h*65+64] = 1
            nc.vector.memset(
                v_sb[:].rearrange("p t (h c) -> p t h c", h=HPC)[:, :, :, 64], 1.0
            )


            # ---------- emission helpers ----------
            def load_chunk(x_sb, x_dram, c):
                nc.sync.dma_start(x_sb[c][:], x_dram.ap()[c])

            NG = NDP if proj_fp8 else NDT

            def qk_proj(x_t, w_sb, b_sb, dst, cc, w, js=(0, 1)):
                """project chunk tile x_t -> dst[:, j, cc:cc+w]."""
                for j in js:
                    ps_p = ps.tile([128, 512], F32, tag="mm", bufs=2, name="ps_p")
                    for g in range(NG):
                        if proj_fp8:
                            nc.tensor.matmul(
                                ps_p[:, 0:w],
                                w_sb[:, g, :, j * 128:(j + 1) * 128],
                                x_t[:, g, :, 0:w],
                                perf_mode=DR,
                                start=(g == 0), stop=(g == NG - 1),
                            )
                        else:
                            nc.tensor.matmul(
                                ps_p[:, 0:w], w_sb[:, g, j * 128:(j + 1) * 128],
                                x_t[:, g, 0:w],
                                start=(g == 0), stop=(g == NG - 1),
                            )
                    nc.vector.tensor_scalar_add(
                        dst[:, j, cc:cc + w], ps_p[:, 0:w], b_sb[:, j:j + 1]
                    )

            def v_proj(kt):
                """value projection for key tile kt -> v_sb[:, kt, :]."""
                x_t = xv_sb[(kt * 128) // VW]
                c0 = (kt * 128) % VW
                ps_v = ps.tile([128, 512], F32, tag="mm", bufs=2, name="ps_v")
                for g in range(NG):
                    if proj_fp8:
                        nc.tensor.matmul(
                            ps_v[:, 0:HL],
                            x_t[:, g, :, c0:c0 + 128],
                            wv_sb[:, g, :, :],
                            perf_mode=DR,
                            start=(g == 0), stop=(g == NG - 1),
                        )
                    else:
                        nc.tensor.matmul(
                            ps_v[:, 0:HL], x_t[:, g, c0:c0 + 128],
                            wv_sb[:, g, :],
                            start=(g == 0), stop=(g == NG - 1),
                        )
                pv4 = ps_v[:, 0:HL].rearrange("p (h c) -> p h c", h=HPC)
                vb4 = bvb_sb.rearrange("p (h c) -> p h c", h=HPC)[:, :, 0:64]
                dst = v_sb[:, kt, :].rearrange("p (h c) -> p h c", h=HPC)[:, :, 0:64]
                nc.vector.tensor_add(dst, pv4, vb4)

            def qk_exp(qb, kb):
                """logits + exp, two heads per Activation instruction."""
                for jp in range(2):  # head pair (jp*2, jp*2+1)
                    lg = ps.tile([128, 1024], F32, tag="lg", bufs=2, name="lg")
                    for hh in range(2):
                        h = jp * 2 + hh
                        p0 = (h % 2) * 64
                        nc.tensor.matmul(
                            lg[:, hh * 512:(hh + 1) * 512],
                            kT[p0:p0 + 64, h // 2, kb * 128:(kb + 1) * 128],
                            qT[p0:p0 + 64, h // 2, qb * 512:(qb + 1) * 512],
                            start=True, stop=True,
                        )
                    nc.scalar.activation(
                        e_sb[:, qb % 2, kb, jp * 1024:(jp + 1) * 1024], lg[:],
                        AFT.Exp, bias=maskb_sb[:, kb:kb + 1], scale=0.125,
                    )

            cn_tiles = {}

            def pv_mm(qb, qq):
                """PV matmuls + normalize for one 128-query chunk."""
                cx = ps.tile([128, HPC * 65], F32, tag="cx", bufs=2, name="cx")
                for h in range(HPC):
                    for kb in range(kblocks):
                        nc.tensor.matmul(
                            cx[:, h * 65:(h + 1) * 65],
                            e_sb[:, qb % 2, kb,
                                 h * 512 + qq * 128:h * 512 + (qq + 1) * 128],
                            v_sb[:, kb, h * 65:(h + 1) * 65],
                            start=(kb == 0), stop=(kb == kblocks - 1),
                        )
                rcp = iop.tile([128, 4], F32, tag="rcp", bufs=3, name="rcp")
                dens = cx[:].rearrange("p (h c) -> p h c", h=HPC)[:, :, 64]
                nc.vector.reciprocal(rcp[:], dens)
                for hp in range(2):
                    cn = iop.tile([128, 128], BF16, tag="cn", bufs=8, name="cn")
                    for hh in range(2):
                        h = hp * 2 + hh
                        nc.vector.tensor_scalar_mul(
                            cn[:, hh * 64:(hh + 1) * 64],
                            cx[:, h * 65:h * 65 + 64], rcp[:, h:h + 1],
                        )
                    cn_tiles[(qq, hp)] = cn

            def pv_tr(qb, qqs):
                """transpose + copy + a2a staging for query chunks."""
                for qq in qqs:
                    ctq = iop.tile([128, 2, 128], cdt, tag="ctq", bufs=4,
                                   name="ctq")
                    for hp in range(2):
                        cn = cn_tiles.pop((qq, hp))
                        # transpose via PE into a bitcast view of an mm tile
                        tp = ps.tile([128, 512], F32, tag="mm", bufs=2, name="tp")
                        tpv = tp[:, 0:64].bitcast(BF16)
                        nc.tensor.transpose(tpv, cn[:], ident_sb[:])
                        if qb == 3:
                            # Act engine is idle after the last exp; keep the
                            # tail's transpose->copy chain off the DVE
                            nc.scalar.copy(ctq[:, hp], tpv)
                        else:
                            nc.vector.tensor_copy(ctq[:, hp], tpv)
                    # stage as two 64-token chunks for the exchange
                    for s in range(2):
                        nc.sync.dma_start(
                            a2a_in[qb].ap()[2 * qq + s],
                            ctq[:, :, s * 64:(s + 1) * 64],
                        )

            def a2a_exchange(j, g0=0, g1=N_CORES):
                if collective:
                    assert (g0, g1) == (0, N_CORES)
                    nc.gpsimd.collective_compute(
                        "AllToAll",
                        mybir.AluOpType.bypass,
                        replica_groups=[list(range(N_CORES))],
                        ins=[a2a_in[j].ap().opt()],
                        outs=[a2a_out[j].ap().opt()],
                    )
                else:
                    nc.gpsimd.dma_start(
                        a2a_out[j].ap()[g0:g1], a2a_in[j].ap()[g0:g1]
                    )

            def ctxf_load(j, halves=(0, 1), ctxf=None, split_q=False):
                # free layout (g, hp, b*64+t): each (g, hp) slice is a valid
                # single-free-dim stationary operand covering both batches
                if ctxf is None:
                    ctxf = iop.tile([128, GROUP, 2, 128], cdt, tag="ctxf",
                                    bufs=2, name="ctxf")
                for b in halves:
                    for hp in range(2):
                        eng = nc.scalar if (split_q and b == 1) else nc.sync
                        eng.dma_start(
                            ctxf[:, :, hp, b * 64:(b + 1) * 64],
                            a2a_out[j].ap()[b * 4:(b + 1) * 4, :, hp].rearrange(
                                "g p t -> p g t"),
                        )
                return ctxf

            def out_proj_bb(j, ctxf, wo_sb, bob_sb):
                """out proj split by batch: batch-0 matmuls only need the
                first four exchanged chunks (ctxf cols 0:64)."""
                o_sb = iop.tile([128, D], BF16, tag="osb", bufs=2, name="o_sb")
                for half in range(2):
                    for b in range(2):
                        ps_o = ps.tile([128, 512], F32, tag="mm", bufs=2,
                                       name="ps_o")
                        n = 0
                        for g in range(GROUP):
                            for hp in range(2):
                                nc.tensor.matmul(
                                    ps_o[0:64, :],
                                    ctxf[:, g, hp, b * 64:(b + 1) * 64],
                                    wo_sb[:, g * 2 + hp,
                                          half * 512:(half + 1) * 512],
                                    start=(n == 0), stop=(n == 2 * GROUP - 1),
                                )
                                n += 1
                        nc.vector.tensor_add(
                            o_sb[b * 64:(b + 1) * 64,
                                 half * 512:(half + 1) * 512],
                            ps_o[0:64, :],
                            bob_sb[0:64, half * 512:(half + 1) * 512],
                        )
                    nc.sync.dma_start(
                        out.ap()[j, :, half * 512:(half + 1) * 512],
                        o_sb[:, half * 512:(half + 1) * 512],
                    )

            def out_proj(j, ctxf, wo_sb, bob_sb, halves=(0, 1), o_sb=None):
                # rows 0:64 = my 64-token slice of batch 0, rows 64:128 batch 1
                if o_sb is None:
                    o_sb = iop.tile([128, D], BF16, tag="osb", bufs=2,
                                    name="o_sb")
                for half in halves:
                    ps_o = ps.tile([128, 512], F32, tag="mm", bufs=2,
                                   name="ps_o")
                    n = 0
                    for g in range(GROUP):
                        for hp in range(2):
                            nc.tensor.matmul(
                                ps_o[:], ctxf[:, g, hp, :],
                                wo_sb[:, g * 2 + hp,
                                      half * 512:(half + 1) * 512],
                                start=(n == 0), stop=(n == 2 * GROUP - 1),
                            )
                            n += 1
                    nc.vector.tensor_add(
                        o_sb[:, half * 512:(half + 1) * 512], ps_o[:],
                        bob_sb[:, half * 512:(half + 1) * 512],
                    )
                    nc.sync.dma_start(
                        out.ap()[j, :, half * 512:(half + 1) * 512],
                        o_sb[:, half * 512:(half + 1) * 512],
                    )
                return o_sb

            # ---------- schedule ----------
            if out_fp8:
                wo_sb = wp.tile([128, GROUP, 2, D], F8)
            else:
                wo_sb = wp.tile([128, NDT, D], BF16)
            bob_sb = wp.tile([128, D], F32)

            for rep in range(reps):
                # ---- round 0: all projections + exp(0) stream.
                # first chunks and weights arrive in interleaved g-halves so
                # the first projection matmuls start ~4us earlier ----
                H0 = NG // 2
                if rep == 0:
                    nc.sync.dma_start(wk_sb[:, 0:H0], wk.ap()[0:H0].rearrange(wre))
                nc.sync.dma_start(xk_sb[0][:, 0:H0], xk.ap()[0, :, 0:H0])
                if rep == 0:
                    nc.sync.dma_start(wq_sb[:, 0:H0], wq.ap()[0:H0].rearrange(wre))
                nc.sync.dma_start(xq_sb[0][:, 0:H0], xq.ap()[0, :, 0:H0])
                if rep == 0:
                    nc.sync.dma_start(const_sb[:], consts.ap())
                    nc.sync.dma_start(wk_sb[:, H0:], wk.ap()[H0:].rearrange(wre))
                nc.sync.dma_start(xk_sb[0][:, H0:], xk.ap()[0, :, H0:])
                if rep == 0:
                    nc.sync.dma_start(wq_sb[:, H0:], wq.ap()[H0:].rearrange(wre))
                nc.sync.dma_start(xq_sb[0][:, H0:], xq.ap()[0, :, H0:])
                qk_proj(xk_sb[0], wk_sb, bk_sb, kT, 0, KW)
                qk_proj(xq_sb[0], wq_sb, bq_sb, qT, 0, 512)
                for kb in range(min(2, kblocks)):
                    qk_exp(0, kb)
                for c in range(1, NKC):
                    load_chunk(xk_sb, xk, c)
                if rep == 0:
                    nc.sync.dma_start(ident_sb[:], ident.ap())
                for c in range(1, NQC):
                    load_chunk(xq_sb, xq, c)
                for c in range(1, NKC):
                    qk_proj(xk_sb[c], wk_sb, bk_sb, kT, c * KW, KW)
                for kb in range(2, min(4, kblocks)):
                    qk_exp(0, kb)
                qk_proj(xq_sb[1], wq_sb, bq_sb, qT, 512, 512)
                for kb in range(4, kblocks):
                    qk_exp(0, kb)
                if rep == 0:
                    nc.sync.dma_start(wv_sb[:], wv.ap().rearrange(wre))
                for c in range(NVC):
                    load_chunk(xv_sb, xv, c)
                for kt in range(min(6, NKT)):
                    v_proj(kt)
                if rep == 0:
                    nc.sync.dma_start(
                        wo_sb[:],
                        wo.ap().rearrange(
                            "g p s m -> p g s m" if out_fp8 else "g p m -> p g m"
                        ),
                    )
                    nc.sync.dma_start(bob_sb[:], bob.ap())

                # ---- rounds 1-3: exp(r) stream hosting round r-1's
                # pv/exchange and round r-2's output projection ----
                ctxf_t = {}
                ob_t = {}
                for r in range(1, NQB):
                    qk_exp(r, 0)
                    if r == 1 and NKT > 6:
                        v_proj(6)
                    qk_exp(r, 1)
                    if r == 1 and NKT > 7:
                        v_proj(7)
                    if r >= 2:
                        cf = ctxf_t[r - 2]
                        ob_t[r - 2] = out_proj(r - 2, cf, wo_sb, bob_sb, (0,))
                    qk_exp(r, 2)
                    if r < NQB - 1:
                        qk_proj(xq_sb[r + 1], wq_sb, bq_sb, qT,
                                (r + 1) * 512, 512, js=(0,))
                    if r == 3:
                        out_proj(0, ctxf_t.pop(0), wo_sb, bob_sb, (1,),
                                 ob_t.pop(0))
                    pv_mm(r - 1, 0)
                    qk_exp(r, 3)
                    if r < NQB - 1:
                        qk_proj(xq_sb[r + 1], wq_sb, bq_sb, qT,
                                (r + 1) * 512, 512, js=(1,))
                    pv_mm(r - 1, 1)
                    pv_tr(r - 1, (0,))
                    qk_exp(r, 4)
                    pv_mm(r - 1, 2)
                    pv_tr(r - 1, (1,))
                    qk_exp(r, 5)
                    pv_mm(r - 1, 3)
                    pv_tr(r - 1, (2,))
                    qk_exp(r, 6)
                    pv_tr(r - 1, (3,))
                    if not collective:
                        a2a_exchange(r - 1, 0, 4)
                    qk_exp(r, 7)
                    if collective:
                        a2a_exchange(r - 1)
                    else:
                        a2a_exchange(r - 1, 4, 8)
                    ctxf_t[r - 1] = ctxf_load(r - 1)

                # ---- tail: last block's pv, then the two outputs ----
                for qq in range(4):
                    pv_mm(3, qq)
                    pv_tr(3, (qq,))
                    if qq == 1 and not collective:
                        a2a_exchange(3, 0, 4)
                out_proj(1, ctxf_t.pop(1), wo_sb, bob_sb, (1,), ob_t.pop(1))
                out_proj(2, ctxf_t.pop(2), wo_sb, bob_sb)
                if collective:
                    a2a_exchange(3)
                else:
                    a2a_exchange(3, 4, 8)
                out_proj(3, ctxf_load(3), wo_sb, bob_sb)

                if dump:
                    nc.sync.dma_start(d_qT.ap(), qT[:])
                    nc.sync.dma_start(d_kT.ap(), kT[:])
                    nc.sync.dma_start(d_v.ap(), v_sb[:])
                    nc.sync.dma_start(d_e.ap(), e_sb[:, 1])

    nc.compile()
    return nc


_NC_CACHE = {}


def _get_nc(key):
    if key not in _NC_CACHE:
        kblocks, proj_fp8, out_fp8 = key
        _NC_CACHE[key] = build_nc(kblocks=kblocks, proj_fp8=proj_fp8,
                                  out_fp8=out_fp8)
    return _NC_CACHE[key]


# identical on every core -> uploaded once, replicated by XLA
_REPLICATED = {"wo", "bob", "ident"}

_RUNNER_CACHE = {}


def _make_runner(nc):
    import jax
    from jax.sharding import Mesh, NamedSharding, PartitionSpec as P
    from jax.experimental.shard_map import shard_map
    import concourse.bass2jax as b2j

    b2j.install_neuronx_cc_hook()
    in_names, out_names, out_avals = [], [], []
    for alloc in nc.m.functions[0].allocations:
        if not isinstance(alloc, mybir.MemoryLocationSet):
            continue
        name = alloc.memorylocations[0].name
        if alloc.kind == "ExternalInput":
            in_names.append(name)
        elif alloc.kind == "ExternalOutput":
            out_names.append(name)
            out_avals.append(
                jax.core.ShapedArray(
                    tuple(alloc.tensor_shape), mybir.dt.np(alloc.dtype)
                )
            )
    pid_name = nc.partition_id_tensor.name if nc.partition_id_tensor else None
    all_in_names = in_names + out_names

    def _body(*args):
        return tuple(
            b2j._bass_exec_p.bind(
                *args,
                out_avals=tuple(out_avals),
                in_names=tuple(all_in_names),
                out_names=tuple(out_names),
                lowering_input_output_aliases=(),
                sim_require_finite=True,
                sim_require_nnan=True,
                nc=nc,
            )
        )

    devices = jax.devices()[:N_CORES]
    mesh = Mesh(np.asarray(devices), ("core",))

    def spec_for(name):
        return P() if name in _REPLICATED else P("core")

    in_specs = tuple(spec_for(n) for n in in_names) + (P("core"),) * len(out_names)
    out_specs = (P("core"),) * len(out_names)
    fn = jax.jit(
        shard_map(_body, mesh=mesh, in_specs=in_specs, out_specs=out_specs,
                  check_rep=False),
        keep_unused=True,
    )
    sh_core = NamedSharding(mesh, P("core"))
    sh_repl = NamedSharding(mesh, P())
    zero_outs = [
        np.zeros((N_CORES * a.shape[0],) + tuple(a.shape[1:]), a.dtype)
        for a in out_avals
    ]
    upload_cache = {}

    def _put(name, arr, sh):
        import hashlib
        key = hashlib.blake2b(arr.tobytes(), digest_size=16).digest()
        hit = upload_cache.get(name)
        if hit is not None and hit[0] == key:
            return hit[1]
        buf = jax.device_put(arr, sh)
        upload_cache[name] = (key, buf)
        return buf

    def run(in_maps):
        args = []
        for name in in_names:
            if name == pid_name:
                cat = np.arange(N_CORES, dtype=np.uint32).reshape(N_CORES, 1)
                args.append(_put(name, cat, sh_core))
            elif name in _REPLICATED:
                args.append(_put(name, np.asarray(in_maps[0][name]), sh_repl))
            else:
                cat = np.concatenate(
                    [np.asarray(m[name]) for m in in_maps], axis=0
                )
                args.append(_put(name, cat, sh_core))
        for i, z in enumerate(zero_outs):
            args.append(_put(f"__zero{i}", z, sh_core))
        outs = fn(*args)
        jax.block_until_ready(outs)
        res = []
        for c in range(N_CORES):
            d = {}
            for i, name in enumerate(out_names):
                arr = np.asarray(outs[i])
                per = arr.shape[0] // N_CORES
                d[name] = arr[c * per:(c + 1) * per]
            res.append(d)
        return res

    return run


def _get_runner(key):
    if key not in _RUNNER_CACHE:
        _RUNNER_CACHE[key] = _make_runner(_get_nc(key))
    return _RUNNER_CACHE[key]


def _dr_pack(a, ncols):
    """[D, n] fp32 -> DoubleRow layout [NDP, 128, 2, n]: row d = 256g+128s+p."""
    return np.ascontiguousarray(a.reshape(NDP, 2, 128, ncols).swapaxes(1, 2))


def _x_chunks(a, W, proj_fp8):
    """[D, n] fp32 -> chunk-major x layout [n//W, 128, ., W]."""
    n = a.shape[1]
    nch = n // W
    if proj_fp8:
        # [g, s, p, c, w] -> [c, p, g, s, w]
        r = a.reshape(NDP, 2, 128, nch, W).transpose(3, 2, 0, 1, 4)
    else:
        r = a.reshape(NDT, 128, nch, W).transpose(2, 1, 0, 3)
    return np.ascontiguousarray(r)


def prepare_in_maps(kblocks, proj_fp8, out_fp8, query, key, value, mask,
                    Wq, bq, Wk, bk, Wv, bv, Wo, bo):
    import ml_dtypes
    bf16 = ml_dtypes.bfloat16
    f8 = ml_dtypes.float8_e4m3
    xnp = f8 if proj_fp8 else bf16
    cnp = f8 if out_fp8 else bf16
    SK = kblocks * 128
    m = np.asarray(mask).reshape(B, S)

    def wpack(a):  # [D, n] fp32 weight -> device layout
        if proj_fp8:
            return _dr_pack(a, a.shape[1]).astype(xnp)
        return np.ascontiguousarray(a.reshape(NDT, 128, a.shape[1])).astype(xnp)

    KW, QW, VW = min(256, SK), 512, min(512, SK)
    xq_b, xk_b, xv_b, maskb_b = [], [], [], []
    for b in range(B):
        idx = np.flatnonzero(m[b] == 0)
        n = len(idx)
        assert n <= SK, f"unmasked count {n} exceeds capacity {SK}"
        k_b = np.zeros((SK, D), np.float32)
        v_b = np.zeros((SK, D), np.float32)
        k_b[:n] = np.asarray(key, np.float32)[b][idx]
        v_b[:n] = np.asarray(value, np.float32)[b][idx]
        xq_b.append(_x_chunks(
            np.ascontiguousarray(np.asarray(query, np.float32)[b].T), QW,
            proj_fp8).astype(xnp))
        xk_b.append(_x_chunks(
            np.ascontiguousarray(k_b.T), KW, proj_fp8).astype(xnp))
        xv_b.append(_x_chunks(
            np.ascontiguousarray(v_b.T), VW, proj_fp8).astype(xnp))
        mb = np.full((kblocks, 128), -1e9, np.float32)
        mb.reshape(-1)[:n] = 0.0
        maskb_b.append(np.ascontiguousarray(mb.T))

    Wo_f = np.asarray(Wo, np.float32)
    if out_fp8:
        Wo_c = np.ascontiguousarray(
            Wo_f.reshape(GROUP, 2, 128, D).swapaxes(1, 2)).astype(cnp)
    else:
        Wo_c = np.ascontiguousarray(Wo_f.reshape(NDT, 128, D)).astype(cnp)
    bob = np.ascontiguousarray(
        np.broadcast_to(np.asarray(bo, np.float32), (128, D)))
    ident = np.eye(128, dtype=np.float32).astype(bf16)

    in_maps = []
    for c in range(N_CORES):
        b, r = c // GROUP, c % GROUP
        sl = slice(r * HL, (r + 1) * HL)
        bv_c = np.asarray(bv, np.float32)[sl]
        bvb = np.zeros((128, HPC * 65), np.float32)
        for h in range(HPC):
            bvb[:, h * 65:h * 65 + 64] = bv_c[h * 64:(h + 1) * 64]
            bvb[:, h * 65 + 64] = 1.0
        consts = np.concatenate([
            np.ascontiguousarray(
                np.asarray(bk, np.float32)[sl].reshape(2, 128).T),
            np.ascontiguousarray(
                np.asarray(bq, np.float32)[sl].reshape(2, 128).T),
            maskb_b[b],
            bvb,
        ], axis=1)
        in_maps.append({
            "xq": xq_b[b], "xk": xk_b[b], "xv": xv_b[b],
            "wq": wpack(np.ascontiguousarray(np.asarray(Wq, np.float32)[:, sl])),
            "wk": wpack(np.ascontiguousarray(np.asarray(Wk, np.float32)[:, sl])),
            "wv": wpack(np.ascontiguousarray(np.asarray(Wv, np.float32)[:, sl])),
            "wo": Wo_c,
            "consts": np.ascontiguousarray(consts),
            "bob": bob,
            "ident": ident,
        })
    return in_maps


def _pick_kblocks(mask):
    m = np.asarray(mask).reshape(B, S)
    maxn = int((m == 0).sum(axis=1).max())
    return min(S // 128, max(1, math.ceil(maxn / 128)))


PROJ_FP8 = False
OUT_FP8 = False


def kernel(**inputs) -> np.ndarray:
    kblocks = _pick_kblocks(inputs["mask"])
    in_maps = prepare_in_maps(kblocks, PROJ_FP8, OUT_FP8, **inputs)
    key = (kblocks, PROJ_FP8, OUT_FP8)
    try:
        run = _get_runner(key)
        results = run(in_maps)
    except Exception:
        res = bass_utils.run_bass_kernel_spmd(
            _get_nc(key), in_maps, core_ids=list(range(N_CORES))
        )
        results = res.results
    out = np.zeros((B, S, D), np.float32)
    for c in range(N_CORES):
        o = np.asarray(results[c]["out"], np.float32)  # [NQB, 128, D]
        for j in range(NQB):
            for beta in range(B):
                out[beta, j * 512 + c * 64:j * 512 + (c + 1) * 64] = \
                    o[j, beta * 64:(beta + 1) * 64]
    return out
